# revision 63
# baseline (speedup 1.0000x reference)
"""Trainium2 Bass kernel for nn_MoEConnectionProcessor (v2).

Data-parallel over cells: 8 cores x 2560 padded cells (19683 real).
Per core: 40 superblocks of 64 cells (1664 edges each).

v2 design (vs v1): minimize PE instruction count / stationary swaps.
  - message projection runs TRANSPOSED: stationary Wm2 (one LDW per
    superblock), moving operand = host-pretransposed nbr^T, pre-masked
    by the functional mask and pre-scaled by 1/cnt_f (relu is positive
    homogeneous, so the scaling commutes through relu).
  - the per-cell term (cur @ Wm1 + b_msg) is added into the same PSUM
    via a 65-row matmul: rows 0..63 = cpm per cell, row 64 = b_msg;
    moving operand = masked staircase built on-device from a host
    weight row (gpsimd partition-broadcast + DVE multiply).
  - functional aggregation = DVE segmented reduce over the 26-edge
    axis of the relu'd transposed messages (no matmul, no masks).
  - local/distant aggregation stays on PE (per-subtile stationary) but
    with host-prebuilt mask*staircase*(1/cnt) moving columns, so
    counts, reciprocals, and mask building all disappear from device.
  - all DMA is contiguous (no DMA-transpose): host prepares both
    layouts of neighbor data.
"""

import numpy as np
import ml_dtypes
from contextlib import ExitStack

N_CELLS, K, D, HG = 19683, 26, 128, 64
NCORES = 8
NS = 2560                 # padded cells per core
SBC = 64                  # cells per superblock
NSB = NS // SBC           # 40 superblocks
NSUB = 13                 # subtiles (128 edges) per superblock
EPB = NSUB * 128          # 1664 edges per superblock
E = NS * K                # 66560 edges per core
NSUBT = NS * K // 128     # 520 subtiles per core
SLOT = 20                 # f-edge slots per cell (max nf in the input)
EPBF = SBC * SLOT         # 1280 f-path edges per superblock
EF = NS * SLOT            # f-path edges per core
FQ = [(0, 512), (512, 512), (1024, 256)]   # f-psum chunks
CPSB = EPBF + EPB         # combo cols per superblock
CHUNK = 512
NCHUNK = NS // CHUNK      # 5
SB_PER_CHUNK = CHUNK // SBC  # 8
CNF_STEPS, DTC = 3, 0.1

bf16 = ml_dtypes.bfloat16

# first local cell of each subtile class (within a 64-cell superblock)
CB_LOC = [(chi * 128) // K for chi in range(NSUB)]


def _consts():
    c = {}
    # S64c [64, EPBF]: staircase indicator for the sorted f-path,
    # cell = e // SLOT (same for all superblocks)
    s64 = np.zeros((SBC, EPBF), np.float32)
    s64[np.arange(EPBF) // SLOT, np.arange(EPBF)] = 1.0
    c["S64c"] = s64.astype(bf16)
    oh = np.zeros((3, 3 * 128), np.float32)
    for m in range(3):
        # distant expert's state is carried as v = 10*s; its gate block
        # absorbs the 0.1
        oh[m, m * 128:(m + 1) * 128] = 0.1 if m == 2 else 1.0
    c["OH3"] = oh.astype(bf16)
    c["ONES3"] = np.ones((3, 1), np.float32).astype(bf16)
    c["ONES164"] = np.ones((1, SBC), np.float32).astype(bf16)
    return c


CONSTS = _consts()


def _build_bass():
    import concourse.bass as bass
    import concourse.tile as tile
    from concourse import bacc, mybir

    f32, bft, i32 = mybir.dt.float32, mybir.dt.bfloat16, mybir.dt.int32
    AF = mybir.ActivationFunctionType
    OP = mybir.AluOpType
    AX = mybir.AxisListType

    nc = bacc.Bacc("TRN2", target_bir_lowering=False, debug=False,
                   num_devices=NCORES)

    def din(name, shape, dt):
        return nc.dram_tensor(name, shape, dt, kind="ExternalInput").ap()

    combo_d = din("combo", [128, NSB * CPSB], bft)  # [natTs | nat] per sb
    anti_d = din("antimask", [1, EF], bft)       # 1.0 on padding f-slots
    invf_d = din("invf_bc", [128, NS], bft)      # 1/cnt_f row-replicated
    Blds_d = din("B_lds", [128, NSUBT * 12], bft)
    S64c_d = din("S64c", [SBC, EPBF], bft)
    bmsgrow_d = din("b_msg_row", [1, D], bft)
    ones164_d = din("ONES164", [1, SBC], bft)
    bigrow_d = din("bigrow", [1, D], bft)        # all-ones row
    curTb_d = din("curT_b", [D, NS], bft)
    curTf_d = din("curT_f", [D, NS], f32)
    wnames = ["Wl1", "Wl2", "Wm1", "Wm2", "Wu1", "Wu2", "Wc1", "Wc2"]
    W = {k: din(k, [D, D], bft) for k in wnames}
    W["Wg1"] = din("Wg1", [D, HG], bft)
    W["Wg2"] = din("Wg2", [HG, 3], bft)
    bias_in = {
        "b_local": din("b_local", [D, 1], f32),
        "b_upd": din("b_upd", [D, 1], f32),
        "b_cnf": din("b_cnf", [D, 1], f32),
        "b_g1": din("b_g1", [HG, 1], f32),
        "b_g2": din("b_g2", [3, 1], f32),
    }
    OH3_d = din("OH3", [3, 384], bft)
    ONES3_d = din("ONES3", [3, 1], bft)
    outT = nc.dram_tensor("outT", [D, NS], f32, kind="ExternalOutput").ap()

    with tile.TileContext(nc) as tc, ExitStack() as ctx:
        const = ctx.enter_context(tc.tile_pool(name="const", bufs=1))
        big = ctx.enter_context(tc.tile_pool(name="big", bufs=1))
        stream = ctx.enter_context(tc.tile_pool(name="stream", bufs=3))
        work = ctx.enter_context(tc.tile_pool(name="work", bufs=2))
        temp1 = ctx.enter_context(tc.tile_pool(name="temp1", bufs=3))
        ps = ctx.enter_context(tc.tile_pool(name="ps", bufs=6, space="PSUM"))
        psagg = ctx.enter_context(tc.tile_pool(name="psagg", bufs=2,
                                               space="PSUM"))

        # ---------- load constants / weights ----------
        # DMA order matters: everything superblock-0 needs comes first so
        # the stream prefetch (combo0) isn't stuck behind cold constants.
        wt = {}
        for k in ("Wm1", "Wm2"):
            t = const.tile([D, D], bft, tag=k)
            nc.sync.dma_start(t[:], W[k][:])
            wt[k] = t
        curTb = const.tile([D, NS], bft)
        nc.sync.dma_start(curTb[:], curTb_d[:])
        # two ping-pong staircase tiles: rows 0..63 = constant staircase,
        # row 64 = per-superblock antimask (streamed by DMA each iteration)
        s64pp = []
        for pi in range(2):
            t_ = const.tile([SBC + 1, EPBF], bft, tag=f"s64pp{pi}")
            nc.sync.dma_start(t_[0:SBC, :], S64c_d[:])
            s64pp.append(t_)
        bmsgrow = const.tile([1, D], bft)
        nc.sync.dma_start(bmsgrow[:], bmsgrow_d[:])
        ones164 = const.tile([1, SBC], bft)
        nc.sync.dma_start(ones164[:], ones164_d[:])
        # cpm ping-pong tiles: rows 0..63 = cur@Wm1 + b_msg per cell
        # (rewritten per superblock), row 64 = -64*ones (loaded once).
        # Paired with the staircase tile (rows 0..63 = cell indicator,
        # row 64 = antimask in {0,1}) the stair matmul adds the per-cell
        # message term AND a -64 penalty on non-functional edges, which
        # the relu turns into exact zeros - no per-edge masking needed.
        cpm_pp = []
        for pi in range(2):
            t_ = const.tile([SBC + 1, D], bft, tag=f"cpm{pi}")
            nc.sync.dma_start(t_[SBC:SBC + 1, :], bigrow_d[:])
            cpm_pp.append(t_)

        aggldT = big.tile([128, NSB * 128], bft)   # col t*128 + 2c + m
        aggfT = big.tile([128, NSB * SBC], bft)    # col t*64 + c
        localT = big.tile([128, NS], bft)
        funcT = big.tile([128, NS], bft)

        def make_cpm(t):
            dst = cpm_pp[t % 2]
            pc = ps.tile([SBC, D], f32, tag="p")
            nc.tensor.matmul(pc[:], curTb[:, t * SBC:(t + 1) * SBC],
                             wt["Wm1"][:], start=True, stop=False)
            nc.tensor.matmul(pc[:], ones164[:], bmsgrow[:],
                             start=False, stop=True)
            nc.scalar.copy(dst[0:SBC, :], pc[:])
            return dst

        def stream_in(t):
            cb_ = stream.tile([128, CPSB], bft, tag="combo")
            nc.sync.dma_start(cb_[:],
                              combo_d[:, t * CPSB:(t + 1) * CPSB])
            nc.sync.dma_start(s64pp[t % 2][SBC:SBC + 1, :],
                              anti_d[:, t * EPBF:(t + 1) * EPBF])
            return cb_

        combo0 = stream_in(0)
        # PE warm-up, gated on the first stream tile so it runs right
        # before superblock 0: ~14 back-to-back 512-col matmuls keep the
        # PE busy >3.4us continuously, opening the HAM clock gate
        # (1.2 -> 2.4 GHz); the main loop's short gaps then never
        # re-throttle it.
        for i in range(14):
            pwu = ps.tile([128, CHUNK], f32, tag="p")
            mm = nc.tensor.matmul(pwu[:], combo0[:, 0:128],
                                  combo0[:, 0:CHUNK], start=True, stop=True)
            if i > 0:
                mm.ins.ldweights = False

        cpm_next = make_cpm(0)

        # gating network: depends only on curTb, so it is emitted INSIDE
        # the main loop (at t==25) where it rides the ACT/DVE slack under
        # the PE-gated superblock cadence instead of serializing the tail.
        g3b = big.tile([3, NS], bft)

        def emit_gating():
            hTg = big.tile([HG, NS], bft)
            for ch in range(NCHUNK):
                sl = slice(ch * CHUNK, (ch + 1) * CHUNK)
                ph = ps.tile([HG, CHUNK], f32, tag="p")
                mm = nc.tensor.matmul(ph[:], wg1[:], curTb[:, sl],
                                      start=True, stop=True)
                if ch > 0:
                    mm.ins.ldweights = False
                nc.scalar.activation(hTg[:, sl], ph[:], AF.Relu,
                                     bias=bias["b_g1"][:])
            e3b = big.tile([3, NS], bft)
            for ch in range(NCHUNK):
                sl = slice(ch * CHUNK, (ch + 1) * CHUNK)
                pz = ps.tile([3, CHUNK], f32, tag="p")
                mm = nc.tensor.matmul(pz[:], wg2[:], hTg[:, sl],
                                      start=True, stop=True)
                if ch > 0:
                    mm.ins.ldweights = False
                nc.scalar.activation(e3b[:, sl], pz[:], AF.Exp,
                                     bias=bias["b_g2"][:])
            lnf = big.tile([1, NS], f32)
            for ch in range(NCHUNK):
                sl = slice(ch * CHUNK, (ch + 1) * CHUNK)
                psum1 = ps.tile([1, CHUNK], f32, tag="p")
                mm = nc.tensor.matmul(psum1[:], ones3[:], e3b[:, sl],
                                      start=True, stop=True)
                if ch > 0:
                    mm.ins.ldweights = False
                nc.scalar.activation(lnf[:, sl], psum1[:], AF.Ln)
            recf = big.tile([1, NS], f32)
            nc.scalar.activation(recf[:], lnf[:], AF.Exp, scale=-1.0)
            # normalized gates: g_m = e_m / den, bf16, broadcast via PE
            rec3 = big.tile([3, NS], f32)
            nc.gpsimd.partition_broadcast(rec3[:], recf[:])
            nc.vector.tensor_tensor(g3b[:], e3b[:], rec3[:], OP.mult)

        # remaining constants (needed mid-superblock-0 or later)
        blds = const.tile([128, NSUBT * 12], bft)
        nc.sync.dma_start(blds[:], Blds_d[:])
        invf = const.tile([128, NS], bft)
        nc.sync.dma_start(invf[:], invf_d[:])
        for k in ("Wl1", "Wl2", "Wu1", "Wu2", "Wc1", "Wc2"):
            t = const.tile([D, D], bft, tag=k)
            nc.sync.dma_start(t[:], W[k][:])
            wt[k] = t
        wg1 = const.tile([D, HG], bft)
        nc.sync.dma_start(wg1[:], W["Wg1"][:])
        wg2 = const.tile([HG, 3], bft)
        nc.sync.dma_start(wg2[:], W["Wg2"][:])
        bias = {}
        for k, ap in bias_in.items():
            t = const.tile(list(ap.shape), f32, tag=k)
            nc.sync.dma_start(t[:], ap[:])
            bias[k] = t
        oh3 = const.tile([3, 384], bft)
        nc.sync.dma_start(oh3[:], OH3_d[:])
        ones3 = const.tile([3, 1], bft)
        nc.sync.dma_start(ones3[:], ONES3_d[:])

        for t in range(NSB):
            cpm_t = cpm_next
            cb_t = combo0 if t == 0 else stream_in(t)
            natT_t = cb_t[:, 0:EPBF]
            nat_t = cb_t[:, EPBF:CPSB]
            s64_t = s64pp[t % 2]

            # messages (transposed, sorted f-slots + penalty):
            # msgsT = relu(Wm2.T @ natTs + cpm @ stair - 64*antimask)
            msgsT = work.tile([128, EPBF], bft, tag="msgs")
            pqs = []
            for q, (q0, qn) in enumerate(FQ):
                pq = ps.tile([128, 512], f32, tag="p")
                pqs.append(pq)
                mm = nc.tensor.matmul(pq[:, 0:qn], wt["Wm2"][:],
                                      natT_t[:, q0:q0 + qn],
                                      start=True, stop=False)
                if q > 0:
                    mm.ins.ldweights = False
            for q, (q0, qn) in enumerate(FQ):
                mm = nc.tensor.matmul(pqs[q][:, 0:qn], cpm_t[:],
                                      s64_t[:, q0:q0 + qn],
                                      start=False, stop=True)
                if q > 0:
                    mm.ins.ldweights = False
                if q < 2:
                    nc.scalar.activation(msgsT[:, q0:q0 + qn],
                                         pqs[q][:, 0:qn], AF.Relu)
                else:
                    nc.vector.tensor_scalar(msgsT[:, q0:q0 + qn],
                                            pqs[q][:, 0:qn], 0.0, None,
                                            OP.max)

            # functional aggregation: segmented sum over the SLOT axis,
            # split as a gpsimd pairwise add (20 -> 10) + a DVE reduce
            # (10 -> 1), then per-cell 1/cnt_f scaling
            mv = msgsT[:].rearrange("p (c k) -> p c k", k=SLOT)
            r1 = work.tile([128, SBC, SLOT // 2], bft, tag="r1")
            nc.gpsimd.tensor_tensor(r1[:], mv[:, :, 0:SLOT // 2],
                                    mv[:, :, SLOT // 2:SLOT], OP.add)
            af = work.tile([128, SBC], f32, tag="af")
            nc.vector.tensor_reduce(af[:], r1[:], AX.X, OP.add)
            nc.vector.tensor_tensor(aggfT[:, t * SBC:(t + 1) * SBC], af[:],
                                    invf[:, t * SBC:(t + 1) * SBC], OP.mult)

            # local/distant aggregation (pre-scaled masked staircase cols)
            pagg = psagg.tile([128, 128], f32, tag="pagg")
            nc.vector.memset(pagg[:], 0.0)
            for sl_ in range(NSUB):
                s = t * NSUB + sl_
                cb = CB_LOC[sl_]
                w2 = 2 * min(6, SBC - cb)
                nc.tensor.matmul(pagg[:, 2 * cb:2 * cb + w2],
                                 nat_t[:, sl_ * 128:(sl_ + 1) * 128],
                                 blds[:, s * 12:s * 12 + w2],
                                 start=False, stop=(sl_ == NSUB - 1))
            nc.scalar.copy(aggldT[:, t * 128:(t + 1) * 128], pagg[:])

            if t == 25:
                emit_gating()
            if t + 1 < NSB:
                cpm_next = make_cpm(t + 1)

        # ---------- second stage (transposed, chunked) ----------
        curTf = const.tile([D, NS], f32)
        nc.sync.dma_start(curTf[:], curTf_d[:])

        def agg_view(off, ch):
            v = aggldT[:, ch * SB_PER_CHUNK * 128 + off:
                       (ch + 1) * SB_PER_CHUNK * 128:2]
            return v.rearrange("p (t c) -> p t c", c=SBC)

        for ch in range(NCHUNK):
            sl = slice(ch * CHUNK, (ch + 1) * CHUNK)
            pl = ps.tile([128, CHUNK], f32, tag="p")
            nc.tensor.matmul(pl[:], wt["Wl1"][:], curTb[:, sl], start=True,
                             stop=False)
            nc.tensor.matmul(
                pl[:].rearrange("p (t c) -> p t c", c=SBC),
                wt["Wl2"][:], agg_view(0, ch), start=False, stop=True)
            nc.scalar.activation(localT[:, sl], pl[:], AF.Tanh,
                                 bias=bias["b_local"][:])
            pf = ps.tile([128, CHUNK], f32, tag="p")
            nc.tensor.matmul(pf[:], wt["Wu1"][:], curTb[:, sl], start=True,
                             stop=False)
            nc.tensor.matmul(pf[:], wt["Wu2"][:], aggfT[:, sl],
                             start=False, stop=True)
            nc.scalar.activation(funcT[:, sl], pf[:], AF.Tanh,
                                 bias=bias["b_upd"][:])

        # CNF: 3 Euler steps; final mix fused into the last step
        s_prev = curTf
        s_prev_bf = curTb
        for step in range(CNF_STEPS):
            last = step == CNF_STEPS - 1
            s_next = big.tile([128, NS], f32, tag=f"s{step % 2}")
            nb_next = None if last else big.tile([128, NS], bft,
                                                 tag=f"sbf{step % 2}")
            for ch in range(NCHUNK):
                sl = slice(ch * CHUNK, (ch + 1) * CHUNK)
                pp = ps.tile([128, CHUNK], f32, tag="p")
                nc.tensor.matmul(pp[:], wt["Wc1"][:], s_prev_bf[:, sl],
                                 start=True, stop=False)
                nc.tensor.matmul(
                    pp[:].rearrange("p (t c) -> p t c", c=SBC),
                    wt["Wc2"][:], agg_view(1, ch), start=False, stop=True)
                # state kept as v = 10*s: the Euler dt folds into the
                # bf16 re-scale copy (scale=0.1) and the gate constant
                th = temp1.tile([128, CHUNK], f32, tag="th")
                nc.scalar.activation(th[:], pp[:], AF.Tanh,
                                     bias=bias["b_cnf"][:])
                nc.vector.tensor_tensor(s_next[:, sl], s_prev[:, sl],
                                        th[:], OP.add)
                if not last:
                    nc.scalar.activation(nb_next[:, sl], s_next[:, sl],
                                         AF.Identity, scale=DTC)
                if last:
                    # final mix for this chunk
                    pe = []
                    for m in range(3):
                        p = ps.tile([128, CHUNK], f32, tag="p")
                        nc.tensor.matmul(p[:],
                                         oh3[:, m * 128:(m + 1) * 128],
                                         g3b[:, sl], start=True, stop=True)
                        pe.append(p)
                    acc = temp1.tile([128, CHUNK], f32, tag="acc")
                    tmp = temp1.tile([128, CHUNK], f32, tag="tmp")
                    nc.vector.tensor_tensor(acc[:], localT[:, sl],
                                            pe[0][:], OP.mult)
                    nc.vector.tensor_tensor(tmp[:], funcT[:, sl],
                                            pe[1][:], OP.mult)
                    nc.vector.tensor_tensor(acc[:], acc[:], tmp[:], OP.add)
                    nc.vector.tensor_tensor(tmp[:], s_next[:, sl],
                                            pe[2][:], OP.mult)
                    nc.vector.tensor_tensor(acc[:], acc[:], tmp[:], OP.add)
                    nc.sync.dma_start(outT[:, sl], acc[:])
            s_prev = s_next
            s_prev_bf = nb_next if step < CNF_STEPS - 1 else None

    nc.compile()
    return nc


_NC_CACHE = None


def _get_nc():
    global _NC_CACHE
    if _NC_CACHE is None:
        _NC_CACHE = _build_bass()
    return _NC_CACHE


def _prep_core_inputs(cur, nbr, conn, weights):
    """cur [NS, D] f32, nbr [NS, K, D] f32, conn [NS, K] i32 -> input map."""
    m = {}
    nf = nbr.reshape(E, D).astype(np.float32)
    connf = conn.reshape(E)
    cellof = np.arange(E) // K
    masks = [(connf == 0), (connf == 2), (connf == 1)]   # l, d, f
    cnts = [np.maximum(mk.reshape(NS, K).sum(1), 1).astype(np.float32)
            for mk in masks]
    # per-edge weights mask/cnt for local/distant
    wl_e = masks[0] / cnts[0][cellof]
    wd_e = masks[1] / cnts[1][cellof]

    # sorted f-path: f-edges first within each cell, truncated to SLOT
    # (exact whenever no cell has more than SLOT functional edges)
    nf_raw = masks[2].reshape(NS, K).sum(1)
    assert nf_raw.max() <= SLOT, f"nf overflow: {nf_raw.max()} > {SLOT}"
    order = np.argsort(~masks[2].reshape(NS, K), axis=1,
                       kind="stable")[:, :SLOT]           # [NS, SLOT]
    natTs = nbr.reshape(NS, K, D)[np.arange(NS)[:, None], order]
    natTs_b = natTs.reshape(NS * SLOT, D).T.reshape(D, NSB, EPBF)
    # combo stream: per superblock [sorted natT block | natural block]
    nat_b = (nf.reshape(NSUBT, 128, D).transpose(1, 0, 2)
             .reshape(128, NSB, EPB))                     # [p, t, (s,d)]
    m["combo"] = np.ascontiguousarray(
        np.concatenate([natTs_b, nat_b], axis=2).reshape(128, NSB * CPSB)
    ).astype(bf16)
    # antimask row: 1.0 on padding f-slots (pairs with the -64 row)
    m["antimask"] = (np.arange(SLOT)[None, :] >= nf_raw[:, None]) \
        .reshape(1, EF).astype(bf16)
    # 1/cnt_f per cell, replicated to 128 partitions
    m["invf_bc"] = np.broadcast_to(
        (1.0 / cnts[2])[None, :], (128, NS)).astype(bf16)
    m["bigrow"] = np.full((1, D), -64.0, np.float32).astype(bf16)

    # B_lds [128, NSUBT*12]: col s*12 + 2*(c_local-cb) + m, pre-scaled
    blds = np.zeros((128, NSUBT * 12), np.float32)
    e_idx = np.arange(E)
    s_idx = e_idx // 128
    p_idx = e_idx % 128
    cb_s = np.array([CB_LOC[si % NSUB] for si in range(NSUBT)])[s_idx]
    j2 = (cellof % SBC) - cb_s
    for mi, we in ((0, wl_e), (1, wd_e)):
        blds[p_idx, s_idx * 12 + 2 * j2 + mi] = we
    m["B_lds"] = blds.astype(bf16)

    ct = np.ascontiguousarray(cur.T)
    m["curT_f"] = (ct * 10.0).astype(np.float32)   # CNF state as 10*s
    m["curT_b"] = ct.astype(bf16)

    Wl, Wm, Wu, Wc = (weights["W_local"], weights["W_msg"],
                      weights["W_upd"], weights["W_cnf"])
    m["Wl1"], m["Wl2"] = Wl[:D].astype(bf16), Wl[D:].astype(bf16)
    m["Wm1"], m["Wm2"] = Wm[:D].astype(bf16), Wm[D:].astype(bf16)
    m["Wu1"], m["Wu2"] = Wu[:D].astype(bf16), Wu[D:].astype(bf16)
    m["Wc1"], m["Wc2"] = Wc[:D].astype(bf16), Wc[D:].astype(bf16)
    m["Wg1"] = weights["W_g1"].astype(bf16)
    m["Wg2"] = weights["W_g2"].astype(bf16)
    m["b_msg_row"] = weights["b_msg"].reshape(1, D).astype(bf16)
    m["b_local"] = weights["b_local"].reshape(D, 1).astype(np.float32)
    m["b_upd"] = weights["b_upd"].reshape(D, 1).astype(np.float32)
    m["b_cnf"] = weights["b_cnf"].reshape(D, 1).astype(np.float32)
    m["b_g1"] = weights["b_g1"].reshape(HG, 1).astype(np.float32)
    m["b_g2"] = weights["b_g2"].reshape(3, 1).astype(np.float32)
    for k, v in CONSTS.items():
        m[k] = v
    return m


def kernel(**inputs):
    from concourse.bass_utils import run_bass_kernel_spmd

    cur = np.asarray(inputs["current_state"], np.float32)
    nbr = np.asarray(inputs["neighbor_states"], np.float32)
    conn = np.asarray(inputs["conn_type"], np.int32)
    weights = {k: np.asarray(v, np.float32) for k, v in inputs.items()
               if k not in ("current_state", "neighbor_states", "conn_type")}

    npad = NCORES * NS
    cur_p = np.zeros((npad, D), np.float32)
    cur_p[:N_CELLS] = cur
    nbr_p = np.zeros((npad, K, D), np.float32)
    nbr_p[:N_CELLS] = nbr
    conn_p = np.full((npad, K), 3, np.int32)
    conn_p[:N_CELLS] = conn

    in_maps = []
    for c in range(NCORES):
        sl = slice(c * NS, (c + 1) * NS)
        in_maps.append(_prep_core_inputs(cur_p[sl], nbr_p[sl], conn_p[sl],
                                         weights))
    nc = _get_nc()
    res = run_bass_kernel_spmd(nc, in_maps, list(range(NCORES)))
    out = np.concatenate([res.results[c]["outT"].T for c in range(NCORES)],
                         axis=0)
    return np.ascontiguousarray(out[:N_CELLS]).astype(np.float32)


if __name__ == "__main__":
    pass


# revision 67
# speedup vs baseline: 1.1166x; 1.1166x over previous
"""Trainium2 Bass kernel for nn_MoEConnectionProcessor (v2).

Data-parallel over cells: 8 cores x 2560 padded cells (19683 real).
Per core: 40 superblocks of 64 cells (1664 edges each).

v2 design (vs v1): minimize PE instruction count / stationary swaps.
  - message projection runs TRANSPOSED: stationary Wm2 (one LDW per
    superblock), moving operand = host-pretransposed nbr^T, pre-masked
    by the functional mask and pre-scaled by 1/cnt_f (relu is positive
    homogeneous, so the scaling commutes through relu).
  - the per-cell term (cur @ Wm1 + b_msg) is added into the same PSUM
    via a 65-row matmul: rows 0..63 = cpm per cell, row 64 = b_msg;
    moving operand = masked staircase built on-device from a host
    weight row (gpsimd partition-broadcast + DVE multiply).
  - functional aggregation = DVE segmented reduce over the 26-edge
    axis of the relu'd transposed messages (no matmul, no masks).
  - local/distant aggregation stays on PE (per-subtile stationary) but
    with host-prebuilt mask*staircase*(1/cnt) moving columns, so
    counts, reciprocals, and mask building all disappear from device.
  - all DMA is contiguous (no DMA-transpose): host prepares both
    layouts of neighbor data.
"""

import numpy as np
import ml_dtypes
from contextlib import ExitStack

N_CELLS, K, D, HG = 19683, 26, 128, 64
NCORES = 8
NS = 2560                 # padded cells per core
SBC = 64                  # cells per superblock
NSB = NS // SBC           # 40 superblocks
NSUB = 13                 # subtiles (128 edges) per superblock
EPB = NSUB * 128          # 1664 edges per superblock
E = NS * K                # 66560 edges per core
NSUBT = NS * K // 128     # 520 subtiles per core
SLOT = 20                 # f-edge slots per cell (max nf in the input)
EPBF = SBC * SLOT         # 1280 f-path edges per superblock
EF = NS * SLOT            # f-path edges per core
FQ = [(0, 512), (512, 512), (1024, 256)]   # f-psum chunks
CPSB = EPBF + EPB         # combo cols per superblock
CHUNK = 512
NCHUNK = NS // CHUNK      # 5
SB_PER_CHUNK = CHUNK // SBC  # 8
CNF_STEPS, DTC = 3, 0.1

bf16 = ml_dtypes.bfloat16

# first local cell of each subtile class (within a 64-cell superblock)
CB_LOC = [(chi * 128) // K for chi in range(NSUB)]


def _consts():
    c = {}
    # S64c [64, EPBF]: staircase indicator for the sorted f-path,
    # cell = e // SLOT (same for all superblocks)
    s64 = np.zeros((SBC, EPBF), np.float32)
    s64[np.arange(EPBF) // SLOT, np.arange(EPBF)] = 1.0
    c["S64c"] = s64.astype(bf16)
    oh = np.zeros((3, 3 * 128), np.float32)
    for m in range(3):
        # distant expert's state is carried as v = 10*s; its gate block
        # absorbs the 0.1
        oh[m, m * 128:(m + 1) * 128] = 0.1 if m == 2 else 1.0
    c["OH3"] = oh.astype(bf16)
    c["ONES3"] = np.ones((3, 1), np.float32).astype(bf16)
    c["ONES164"] = np.ones((1, SBC), np.float32).astype(bf16)
    return c


CONSTS = _consts()


def _build_bass():
    import concourse.bass as bass
    import concourse.tile as tile
    from concourse import bacc, mybir

    f32, bft, i32 = mybir.dt.float32, mybir.dt.bfloat16, mybir.dt.int32
    AF = mybir.ActivationFunctionType
    OP = mybir.AluOpType
    AX = mybir.AxisListType

    nc = bacc.Bacc("TRN2", target_bir_lowering=False, debug=False,
                   num_devices=NCORES)

    def din(name, shape, dt):
        return nc.dram_tensor(name, shape, dt, kind="ExternalInput").ap()

    combo_d = din("combo", [128, NSB * CPSB], bft)  # [natTs | nat] per sb
    anti_d = din("antimask", [1, EF], bft)       # 1.0 on padding f-slots
    invf_d = din("invf_bc", [128, NS], bft)      # 1/cnt_f row-replicated
    Blds_d = din("B_lds", [128, NSUBT * 12], bft)
    S64c_d = din("S64c", [SBC, EPBF], bft)
    bmsgrow_d = din("b_msg_row", [1, D], bft)
    ones164_d = din("ONES164", [1, SBC], bft)
    bigrow_d = din("bigrow", [1, D], bft)        # all-ones row
    curTb_d = din("curT_b", [D, NS], bft)
    curTf_d = din("curT_f", [D, NS], f32)
    wnames = ["Wl1", "Wl2", "Wm1", "Wm2", "Wu1", "Wu2", "Wc1", "Wc2"]
    W = {k: din(k, [D, D], bft) for k in wnames}
    W["Wg1"] = din("Wg1", [D, HG], bft)
    W["Wg2"] = din("Wg2", [HG, 3], bft)
    bias_in = {
        "b_local": din("b_local", [D, 1], f32),
        "b_upd": din("b_upd", [D, 1], f32),
        "b_cnf": din("b_cnf", [D, 1], f32),
        "b_g1": din("b_g1", [HG, 1], f32),
        "b_g2": din("b_g2", [3, 1], f32),
    }
    OH3_d = din("OH3", [3, 384], bft)
    ONES3_d = din("ONES3", [3, 1], bft)
    outT = nc.dram_tensor("outT", [D, NS], f32, kind="ExternalOutput").ap()

    with tile.TileContext(nc) as tc, ExitStack() as ctx:
        const = ctx.enter_context(tc.tile_pool(name="const", bufs=1))
        big = ctx.enter_context(tc.tile_pool(name="big", bufs=1))
        stream = ctx.enter_context(tc.tile_pool(name="stream", bufs=4))
        work = ctx.enter_context(tc.tile_pool(name="work", bufs=2))
        temp1 = ctx.enter_context(tc.tile_pool(name="temp1", bufs=3))
        ps = ctx.enter_context(tc.tile_pool(name="ps", bufs=6, space="PSUM"))
        psagg = ctx.enter_context(tc.tile_pool(name="psagg", bufs=2,
                                               space="PSUM"))

        # ---------- load constants / weights ----------
        # DMA order matters: combo0 (warm-up gate + superblock 0's data)
        # is the very first trigger; everything else loads behind it.
        s64pp = []
        for pi in range(2):
            t_ = const.tile([SBC + 1, EPBF], bft, tag=f"s64pp{pi}")
            s64pp.append(t_)

        def stream_in(t):
            cb_ = stream.tile([128, CPSB], bft, tag="combo")
            nc.sync.dma_start(cb_[:],
                              combo_d[:, t * CPSB:(t + 1) * CPSB])
            nc.sync.dma_start(s64pp[t % 2][SBC:SBC + 1, :],
                              anti_d[:, t * EPBF:(t + 1) * EPBF])
            return cb_

        combo0 = stream_in(0)
        # PE warm-up, gated on the first stream tile so it runs right
        # before superblock 0: ~14 back-to-back 512-col matmuls keep the
        # PE busy >3.4us continuously, opening the HAM clock gate
        # (1.2 -> 2.4 GHz).
        for i in range(14):
            pwu = ps.tile([128, CHUNK], f32, tag="p")
            mm = nc.tensor.matmul(pwu[:], combo0[:, 0:128],
                                  combo0[:, 0:CHUNK], start=True, stop=True)
            if i > 0:
                mm.ins.ldweights = False

        wt = {}
        for k in ("Wm1", "Wm2"):
            t = const.tile([D, D], bft, tag=k)
            nc.sync.dma_start(t[:], W[k][:])
            wt[k] = t
        curTb = const.tile([D, NS], bft)
        nc.sync.dma_start(curTb[:], curTb_d[:])
        for pi in range(2):
            nc.sync.dma_start(s64pp[pi][0:SBC, :], S64c_d[:])
        bmsgrow = const.tile([1, D], bft)
        nc.sync.dma_start(bmsgrow[:], bmsgrow_d[:])
        ones164 = const.tile([1, SBC], bft)
        nc.sync.dma_start(ones164[:], ones164_d[:])
        # cpm ping-pong tiles: rows 0..63 = cur@Wm1 + b_msg per cell
        # (rewritten per superblock), row 64 = -64*ones (loaded once).
        # Paired with the staircase tile (rows 0..63 = cell indicator,
        # row 64 = antimask in {0,1}) the stair matmul adds the per-cell
        # message term AND a -64 penalty on non-functional edges, which
        # the relu turns into exact zeros - no per-edge masking needed.
        cpm_pp = []
        for pi in range(2):
            t_ = const.tile([SBC + 1, D], bft, tag=f"cpm{pi}")
            nc.sync.dma_start(t_[SBC:SBC + 1, :], bigrow_d[:])
            cpm_pp.append(t_)

        aggldT = big.tile([128, NSB * 128], bft)   # col t*128 + 2c + m
        aggfT = big.tile([128, NSB * SBC], bft)    # col t*64 + c
        localT = big.tile([128, NS], bft)
        funcT = big.tile([128, NS], bft)

        def make_cpm(t):
            dst = cpm_pp[t % 2]
            pc = ps.tile([SBC, D], f32, tag="p")
            nc.tensor.matmul(pc[:], curTb[:, t * SBC:(t + 1) * SBC],
                             wt["Wm1"][:], start=True, stop=False)
            nc.tensor.matmul(pc[:], ones164[:], bmsgrow[:],
                             start=False, stop=True)
            nc.scalar.copy(dst[0:SBC, :], pc[:])
            return dst

        cpm_next = make_cpm(0)

        # remaining constants (needed mid-superblock-0 or later)
        blds = const.tile([128, NSUBT * 12], bft)
        nc.sync.dma_start(blds[:], Blds_d[:])
        invf = const.tile([128, NS], bft)
        nc.sync.dma_start(invf[:], invf_d[:])
        for k in ("Wl1", "Wl2", "Wu1", "Wu2", "Wc1", "Wc2"):
            t = const.tile([D, D], bft, tag=k)
            nc.sync.dma_start(t[:], W[k][:])
            wt[k] = t
        wg1 = const.tile([D, HG], bft)
        nc.sync.dma_start(wg1[:], W["Wg1"][:])
        wg2 = const.tile([HG, 3], bft)
        nc.sync.dma_start(wg2[:], W["Wg2"][:])
        bias = {}
        for k, ap in bias_in.items():
            t = const.tile(list(ap.shape), f32, tag=k)
            nc.sync.dma_start(t[:], ap[:])
            bias[k] = t
        oh3 = const.tile([3, 384], bft)
        nc.sync.dma_start(oh3[:], OH3_d[:])
        ones3 = const.tile([3, 1], bft)
        nc.sync.dma_start(ones3[:], ONES3_d[:])

        for t in range(NSB):
            cpm_t = cpm_next
            cb_t = combo0 if t == 0 else stream_in(t)
            natT_t = cb_t[:, 0:EPBF]
            nat_t = cb_t[:, EPBF:CPSB]
            s64_t = s64pp[t % 2]

            # messages (transposed, sorted f-slots + penalty):
            # msgsT = relu(Wm2.T @ natTs + cpm @ stair - 64*antimask)
            msgsT = work.tile([128, EPBF], bft, tag="msgs")
            pqs = []
            for q, (q0, qn) in enumerate(FQ):
                pq = ps.tile([128, 512], f32, tag="p")
                pqs.append(pq)
                mm = nc.tensor.matmul(pq[:, 0:qn], wt["Wm2"][:],
                                      natT_t[:, q0:q0 + qn],
                                      start=True, stop=False)
                if q > 0:
                    mm.ins.ldweights = False
            for q, (q0, qn) in enumerate(FQ):
                mm = nc.tensor.matmul(pqs[q][:, 0:qn], cpm_t[:],
                                      s64_t[:, q0:q0 + qn],
                                      start=False, stop=True)
                if q > 0:
                    mm.ins.ldweights = False
                if q < 2:
                    nc.scalar.activation(msgsT[:, q0:q0 + qn],
                                         pqs[q][:, 0:qn], AF.Relu)
                else:
                    nc.vector.tensor_scalar(msgsT[:, q0:q0 + qn],
                                            pqs[q][:, 0:qn], 0.0, None,
                                            OP.max)

            # functional aggregation: segmented sum over the SLOT axis,
            # split as a gpsimd pairwise add (20 -> 10) + a DVE reduce
            # (10 -> 1), then per-cell 1/cnt_f scaling
            mv = msgsT[:].rearrange("p (c k) -> p c k", k=SLOT)
            r1 = work.tile([128, SBC, SLOT // 2], bft, tag="r1")
            nc.gpsimd.tensor_tensor(r1[:], mv[:, :, 0:SLOT // 2],
                                    mv[:, :, SLOT // 2:SLOT], OP.add)
            af = work.tile([128, SBC], f32, tag="af")
            nc.vector.tensor_reduce(af[:], r1[:], AX.X, OP.add)
            nc.vector.tensor_tensor(aggfT[:, t * SBC:(t + 1) * SBC], af[:],
                                    invf[:, t * SBC:(t + 1) * SBC], OP.mult)

            # local/distant aggregation (pre-scaled masked staircase cols)
            pagg = psagg.tile([128, 128], f32, tag="pagg")
            nc.vector.memset(pagg[:], 0.0)
            for sl_ in range(NSUB):
                s = t * NSUB + sl_
                cb = CB_LOC[sl_]
                w2 = 2 * min(6, SBC - cb)
                nc.tensor.matmul(pagg[:, 2 * cb:2 * cb + w2],
                                 nat_t[:, sl_ * 128:(sl_ + 1) * 128],
                                 blds[:, s * 12:s * 12 + w2],
                                 start=False, stop=(sl_ == NSUB - 1))
            nc.scalar.copy(aggldT[:, t * 128:(t + 1) * 128], pagg[:])

            if t + 1 < NSB:
                cpm_next = make_cpm(t + 1)

        # ---------- second stage (transposed, chunked) ----------
        # order: gating first (Relu table is already loaded from the main
        # loop), then local/func (Tanh), then CNF with the final mix fused
        # into the last Euler step.
        curTf = const.tile([D, NS], f32)
        nc.sync.dma_start(curTf[:], curTf_d[:])

        def agg_view(off, ch):
            v = aggldT[:, ch * SB_PER_CHUNK * 128 + off:
                       (ch + 1) * SB_PER_CHUNK * 128:2]
            return v.rearrange("p (t c) -> p t c", c=SBC)

        hTg = big.tile([HG, NS], bft)
        for ch in range(NCHUNK):
            sl = slice(ch * CHUNK, (ch + 1) * CHUNK)
            ph = ps.tile([HG, CHUNK], f32, tag="p")
            mm = nc.tensor.matmul(ph[:], wg1[:], curTb[:, sl], start=True,
                                  stop=True)
            if ch > 0:
                mm.ins.ldweights = False
            nc.scalar.activation(hTg[:, sl], ph[:], AF.Relu,
                                 bias=bias["b_g1"][:])
        e3b = big.tile([3, NS], bft)
        for ch in range(NCHUNK):
            sl = slice(ch * CHUNK, (ch + 1) * CHUNK)
            pz = ps.tile([3, CHUNK], f32, tag="p")
            mm = nc.tensor.matmul(pz[:], wg2[:], hTg[:, sl], start=True,
                                  stop=True)
            if ch > 0:
                mm.ins.ldweights = False
            nc.scalar.activation(e3b[:, sl], pz[:], AF.Exp,
                                 bias=bias["b_g2"][:])
        lnf = big.tile([1, NS], f32)
        for ch in range(NCHUNK):
            sl = slice(ch * CHUNK, (ch + 1) * CHUNK)
            psum1 = ps.tile([1, CHUNK], f32, tag="p")
            mm = nc.tensor.matmul(psum1[:], ones3[:], e3b[:, sl], start=True,
                                  stop=True)
            if ch > 0:
                mm.ins.ldweights = False
            nc.scalar.activation(lnf[:, sl], psum1[:], AF.Ln)
        recf = big.tile([1, NS], f32)
        nc.scalar.activation(recf[:], lnf[:], AF.Exp, scale=-1.0)
        # normalized gates: g_m = e_m / den, in bf16, broadcast via PE
        rec3 = big.tile([3, NS], f32)
        nc.gpsimd.partition_broadcast(rec3[:], recf[:])
        g3b = big.tile([3, NS], bft)
        nc.vector.tensor_tensor(g3b[:], e3b[:], rec3[:], OP.mult)

        for ch in range(NCHUNK):
            sl = slice(ch * CHUNK, (ch + 1) * CHUNK)
            pl = ps.tile([128, CHUNK], f32, tag="p")
            nc.tensor.matmul(pl[:], wt["Wl1"][:], curTb[:, sl], start=True,
                             stop=False)
            nc.tensor.matmul(
                pl[:].rearrange("p (t c) -> p t c", c=SBC),
                wt["Wl2"][:], agg_view(0, ch), start=False, stop=True)
            nc.scalar.activation(localT[:, sl], pl[:], AF.Tanh,
                                 bias=bias["b_local"][:])
            pf = ps.tile([128, CHUNK], f32, tag="p")
            nc.tensor.matmul(pf[:], wt["Wu1"][:], curTb[:, sl], start=True,
                             stop=False)
            nc.tensor.matmul(pf[:], wt["Wu2"][:], aggfT[:, sl],
                             start=False, stop=True)
            nc.scalar.activation(funcT[:, sl], pf[:], AF.Tanh,
                                 bias=bias["b_upd"][:])

        # CNF: 3 Euler steps; final mix fused into the last step
        s_prev = curTf
        s_prev_bf = curTb
        for step in range(CNF_STEPS):
            last = step == CNF_STEPS - 1
            s_next = big.tile([128, NS], f32, tag=f"s{step % 2}")
            nb_next = None if last else big.tile([128, NS], bft,
                                                 tag=f"sbf{step % 2}")
            for ch in range(NCHUNK):
                sl = slice(ch * CHUNK, (ch + 1) * CHUNK)
                pp = ps.tile([128, CHUNK], f32, tag="p")
                nc.tensor.matmul(pp[:], wt["Wc1"][:], s_prev_bf[:, sl],
                                 start=True, stop=False)
                nc.tensor.matmul(
                    pp[:].rearrange("p (t c) -> p t c", c=SBC),
                    wt["Wc2"][:], agg_view(1, ch), start=False, stop=True)
                # state kept as v = 10*s: the Euler dt folds into the
                # bf16 re-scale copy (scale=0.1) and the gate constant
                th = temp1.tile([128, CHUNK], f32, tag="th")
                nc.scalar.activation(th[:], pp[:], AF.Tanh,
                                     bias=bias["b_cnf"][:])
                nc.vector.tensor_tensor(s_next[:, sl], s_prev[:, sl],
                                        th[:], OP.add)
                if not last:
                    nc.scalar.activation(nb_next[:, sl], s_next[:, sl],
                                         AF.Identity, scale=DTC)
                if last:
                    # final mix for this chunk
                    pe = []
                    for m in range(3):
                        p = ps.tile([128, CHUNK], f32, tag="p")
                        nc.tensor.matmul(p[:],
                                         oh3[:, m * 128:(m + 1) * 128],
                                         g3b[:, sl], start=True, stop=True)
                        pe.append(p)
                    acc = temp1.tile([128, CHUNK], f32, tag="acc")
                    tmp = temp1.tile([128, CHUNK], f32, tag="tmp")
                    nc.vector.tensor_tensor(acc[:], localT[:, sl],
                                            pe[0][:], OP.mult)
                    nc.vector.tensor_tensor(tmp[:], funcT[:, sl],
                                            pe[1][:], OP.mult)
                    nc.vector.tensor_tensor(acc[:], acc[:], tmp[:], OP.add)
                    nc.vector.tensor_tensor(tmp[:], s_next[:, sl],
                                            pe[2][:], OP.mult)
                    nc.vector.tensor_tensor(acc[:], acc[:], tmp[:], OP.add)
                    nc.sync.dma_start(outT[:, sl], acc[:])
            s_prev = s_next
            s_prev_bf = nb_next if step < CNF_STEPS - 1 else None

    nc.compile()
    return nc


_NC_CACHE = None


def _get_nc():
    global _NC_CACHE
    if _NC_CACHE is None:
        _NC_CACHE = _build_bass()
    return _NC_CACHE


def _prep_core_inputs(cur, nbr, conn, weights):
    """cur [NS, D] f32, nbr [NS, K, D] f32, conn [NS, K] i32 -> input map."""
    m = {}
    nf = nbr.reshape(E, D).astype(np.float32)
    connf = conn.reshape(E)
    cellof = np.arange(E) // K
    masks = [(connf == 0), (connf == 2), (connf == 1)]   # l, d, f
    cnts = [np.maximum(mk.reshape(NS, K).sum(1), 1).astype(np.float32)
            for mk in masks]
    # per-edge weights mask/cnt for local/distant
    wl_e = masks[0] / cnts[0][cellof]
    wd_e = masks[1] / cnts[1][cellof]

    # sorted f-path: f-edges first within each cell, truncated to SLOT
    # (exact whenever no cell has more than SLOT functional edges)
    nf_raw = masks[2].reshape(NS, K).sum(1)
    assert nf_raw.max() <= SLOT, f"nf overflow: {nf_raw.max()} > {SLOT}"
    order = np.argsort(~masks[2].reshape(NS, K), axis=1,
                       kind="stable")[:, :SLOT]           # [NS, SLOT]
    natTs = nbr.reshape(NS, K, D)[np.arange(NS)[:, None], order]
    natTs_b = natTs.reshape(NS * SLOT, D).T.reshape(D, NSB, EPBF)
    # combo stream: per superblock [sorted natT block | natural block]
    nat_b = (nf.reshape(NSUBT, 128, D).transpose(1, 0, 2)
             .reshape(128, NSB, EPB))                     # [p, t, (s,d)]
    m["combo"] = np.ascontiguousarray(
        np.concatenate([natTs_b, nat_b], axis=2).reshape(128, NSB * CPSB)
    ).astype(bf16)
    # antimask row: 1.0 on padding f-slots (pairs with the -64 row)
    m["antimask"] = (np.arange(SLOT)[None, :] >= nf_raw[:, None]) \
        .reshape(1, EF).astype(bf16)
    # 1/cnt_f per cell, replicated to 128 partitions
    m["invf_bc"] = np.broadcast_to(
        (1.0 / cnts[2])[None, :], (128, NS)).astype(bf16)
    m["bigrow"] = np.full((1, D), -64.0, np.float32).astype(bf16)

    # B_lds [128, NSUBT*12]: col s*12 + 2*(c_local-cb) + m, pre-scaled
    blds = np.zeros((128, NSUBT * 12), np.float32)
    e_idx = np.arange(E)
    s_idx = e_idx // 128
    p_idx = e_idx % 128
    cb_s = np.array([CB_LOC[si % NSUB] for si in range(NSUBT)])[s_idx]
    j2 = (cellof % SBC) - cb_s
    for mi, we in ((0, wl_e), (1, wd_e)):
        blds[p_idx, s_idx * 12 + 2 * j2 + mi] = we
    m["B_lds"] = blds.astype(bf16)

    ct = np.ascontiguousarray(cur.T)
    m["curT_f"] = (ct * 10.0).astype(np.float32)   # CNF state as 10*s
    m["curT_b"] = ct.astype(bf16)

    Wl, Wm, Wu, Wc = (weights["W_local"], weights["W_msg"],
                      weights["W_upd"], weights["W_cnf"])
    m["Wl1"], m["Wl2"] = Wl[:D].astype(bf16), Wl[D:].astype(bf16)
    m["Wm1"], m["Wm2"] = Wm[:D].astype(bf16), Wm[D:].astype(bf16)
    m["Wu1"], m["Wu2"] = Wu[:D].astype(bf16), Wu[D:].astype(bf16)
    m["Wc1"], m["Wc2"] = Wc[:D].astype(bf16), Wc[D:].astype(bf16)
    m["Wg1"] = weights["W_g1"].astype(bf16)
    m["Wg2"] = weights["W_g2"].astype(bf16)
    m["b_msg_row"] = weights["b_msg"].reshape(1, D).astype(bf16)
    m["b_local"] = weights["b_local"].reshape(D, 1).astype(np.float32)
    m["b_upd"] = weights["b_upd"].reshape(D, 1).astype(np.float32)
    m["b_cnf"] = weights["b_cnf"].reshape(D, 1).astype(np.float32)
    m["b_g1"] = weights["b_g1"].reshape(HG, 1).astype(np.float32)
    m["b_g2"] = weights["b_g2"].reshape(3, 1).astype(np.float32)
    for k, v in CONSTS.items():
        m[k] = v
    return m


def kernel(**inputs):
    from concourse.bass_utils import run_bass_kernel_spmd

    cur = np.asarray(inputs["current_state"], np.float32)
    nbr = np.asarray(inputs["neighbor_states"], np.float32)
    conn = np.asarray(inputs["conn_type"], np.int32)
    weights = {k: np.asarray(v, np.float32) for k, v in inputs.items()
               if k not in ("current_state", "neighbor_states", "conn_type")}

    npad = NCORES * NS
    cur_p = np.zeros((npad, D), np.float32)
    cur_p[:N_CELLS] = cur
    nbr_p = np.zeros((npad, K, D), np.float32)
    nbr_p[:N_CELLS] = nbr
    conn_p = np.full((npad, K), 3, np.int32)
    conn_p[:N_CELLS] = conn

    in_maps = []
    for c in range(NCORES):
        sl = slice(c * NS, (c + 1) * NS)
        in_maps.append(_prep_core_inputs(cur_p[sl], nbr_p[sl], conn_p[sl],
                                         weights))
    nc = _get_nc()
    res = run_bass_kernel_spmd(nc, in_maps, list(range(NCORES)))
    out = np.concatenate([res.results[c]["outT"].T for c in range(NCORES)],
                         axis=0)
    return np.ascontiguousarray(out[:N_CELLS]).astype(np.float32)


if __name__ == "__main__":
    pass


# revision 69
# speedup vs baseline: 1.1745x; 1.0518x over previous
"""Trainium2 Bass kernel for nn_MoEConnectionProcessor (v2).

Data-parallel over cells: 8 cores x 2560 padded cells (19683 real).
Per core: 40 superblocks of 64 cells (1664 edges each).

v2 design (vs v1): minimize PE instruction count / stationary swaps.
  - message projection runs TRANSPOSED: stationary Wm2 (one LDW per
    superblock), moving operand = host-pretransposed nbr^T, pre-masked
    by the functional mask and pre-scaled by 1/cnt_f (relu is positive
    homogeneous, so the scaling commutes through relu).
  - the per-cell term (cur @ Wm1 + b_msg) is added into the same PSUM
    via a 65-row matmul: rows 0..63 = cpm per cell, row 64 = b_msg;
    moving operand = masked staircase built on-device from a host
    weight row (gpsimd partition-broadcast + DVE multiply).
  - functional aggregation = DVE segmented reduce over the 26-edge
    axis of the relu'd transposed messages (no matmul, no masks).
  - local/distant aggregation stays on PE (per-subtile stationary) but
    with host-prebuilt mask*staircase*(1/cnt) moving columns, so
    counts, reciprocals, and mask building all disappear from device.
  - all DMA is contiguous (no DMA-transpose): host prepares both
    layouts of neighbor data.
"""

import numpy as np
import ml_dtypes
from contextlib import ExitStack

N_CELLS, K, D, HG = 19683, 26, 128, 64
NCORES = 8
NS = 2560                 # padded cells per core
SBC = 64                  # cells per superblock
NSB = NS // SBC           # 40 superblocks
NSUB = 13                 # subtiles (128 edges) per superblock
EPB = NSUB * 128          # 1664 edges per superblock
E = NS * K                # 66560 edges per core
NSUBT = NS * K // 128     # 520 subtiles per core
SLOT = 20                 # f-edge slots per cell (max nf in the input)
EPBF = SBC * SLOT         # 1280 f-path edges per superblock
EF = NS * SLOT            # f-path edges per core
FQ = [(0, 512), (512, 512), (1024, 256)]   # f-psum chunks
CPSB = EPBF + EPB         # combo cols per superblock
CHUNK = 512
NCHUNK = NS // CHUNK      # 5
SB_PER_CHUNK = CHUNK // SBC  # 8
CNF_STEPS, DTC = 3, 0.1

bf16 = ml_dtypes.bfloat16

# first local cell of each subtile class (within a 64-cell superblock)
CB_LOC = [(chi * 128) // K for chi in range(NSUB)]


def _consts():
    c = {}
    # S64c [64, EPBF]: staircase indicator for the sorted f-path,
    # cell = e // SLOT (same for all superblocks)
    s64 = np.zeros((SBC, EPBF), np.float32)
    s64[np.arange(EPBF) // SLOT, np.arange(EPBF)] = 1.0
    c["S64c"] = s64.astype(bf16)
    oh = np.zeros((3, 3 * 128), np.float32)
    for m in range(3):
        # distant expert's state is carried as v = 10*s; its gate block
        # absorbs the 0.1
        oh[m, m * 128:(m + 1) * 128] = 0.1 if m == 2 else 1.0
    c["OH3"] = oh.astype(bf16)
    c["ONES3"] = np.ones((3, 1), np.float32).astype(bf16)
    c["ONES164"] = np.ones((1, SBC), np.float32).astype(bf16)
    return c


CONSTS = _consts()


def _build_bass():
    import concourse.bass as bass
    import concourse.tile as tile
    from concourse import bacc, mybir

    f32, bft, i32 = mybir.dt.float32, mybir.dt.bfloat16, mybir.dt.int32
    AF = mybir.ActivationFunctionType
    OP = mybir.AluOpType
    AX = mybir.AxisListType

    nc = bacc.Bacc("TRN2", target_bir_lowering=False, debug=False,
                   num_devices=NCORES)

    def din(name, shape, dt):
        return nc.dram_tensor(name, shape, dt, kind="ExternalInput").ap()

    combo_d = din("combo", [128, NSB * CPSB], bft)  # [natTs | nat] per sb
    anti_d = din("antimask", [1, EF], bft)       # 1.0 on padding f-slots
    invf_d = din("invf_bc", [128, NS], bft)      # 1/cnt_f row-replicated
    Blds_d = din("B_lds", [128, NSUBT * 12], bft)
    S64c_d = din("S64c", [SBC, EPBF], bft)
    bmsgrow_d = din("b_msg_row", [1, D], bft)
    ones164_d = din("ONES164", [1, SBC], bft)
    bigrow_d = din("bigrow", [1, D], bft)        # all-ones row
    curTb_d = din("curT_b", [D, NS], bft)
    curTf_d = din("curT_f", [D, NS], f32)
    wnames = ["Wl1", "Wl2", "Wm1", "Wm2", "Wu1", "Wu2", "Wc1", "Wc2"]
    W = {k: din(k, [D, D], bft) for k in wnames}
    W["Wg1"] = din("Wg1", [D, HG], bft)
    W["Wg2"] = din("Wg2", [HG, 3], bft)
    bias_in = {
        "b_local": din("b_local", [D, 1], f32),
        "b_upd": din("b_upd", [D, 1], f32),
        "b_cnf": din("b_cnf", [D, 1], f32),
        "b_g1": din("b_g1", [HG, 1], f32),
        "b_g2": din("b_g2", [3, 1], f32),
    }
    OH3_d = din("OH3", [3, 384], bft)
    ONES3_d = din("ONES3", [3, 1], bft)
    outT = nc.dram_tensor("outT", [D, NS], f32, kind="ExternalOutput").ap()

    with tile.TileContext(nc) as tc, ExitStack() as ctx:
        const = ctx.enter_context(tc.tile_pool(name="const", bufs=1))
        big = ctx.enter_context(tc.tile_pool(name="big", bufs=1))
        stream = ctx.enter_context(tc.tile_pool(name="stream", bufs=4))
        work = ctx.enter_context(tc.tile_pool(name="work", bufs=2))
        temp1 = ctx.enter_context(tc.tile_pool(name="temp1", bufs=3))
        ps = ctx.enter_context(tc.tile_pool(name="ps", bufs=6, space="PSUM"))
        psagg = ctx.enter_context(tc.tile_pool(name="psagg", bufs=2,
                                               space="PSUM"))

        # ---------- load constants / weights ----------
        # DMA order matters: combo0 (warm-up gate + superblock 0's data)
        # is the very first trigger; everything else loads behind it.
        s64pp = []
        for pi in range(2):
            t_ = const.tile([SBC + 1, EPBF], bft, tag=f"s64pp{pi}")
            s64pp.append(t_)

        def stream_in(t):
            cb_ = stream.tile([128, CPSB], bft, tag="combo")
            nc.sync.dma_start(cb_[:],
                              combo_d[:, t * CPSB:(t + 1) * CPSB])
            nc.sync.dma_start(s64pp[t % 2][SBC:SBC + 1, :],
                              anti_d[:, t * EPBF:(t + 1) * EPBF])
            return cb_

        combo0 = stream_in(0)
        # PE warm-up, gated on the first stream tile so it runs right
        # before superblock 0: ~14 back-to-back 512-col matmuls keep the
        # PE busy >3.4us continuously, opening the HAM clock gate
        # (1.2 -> 2.4 GHz).
        for i in range(14):
            pwu = ps.tile([128, CHUNK], f32, tag="p")
            mm = nc.tensor.matmul(pwu[:], combo0[:, 0:128],
                                  combo0[:, 0:CHUNK], start=True, stop=True)
            if i > 0:
                mm.ins.ldweights = False

        wt = {}
        for k in ("Wm1", "Wm2"):
            t = const.tile([D, D], bft, tag=k)
            nc.sync.dma_start(t[:], W[k][:])
            wt[k] = t
        curTb = const.tile([D, NS], bft)
        nc.sync.dma_start(curTb[:], curTb_d[:])
        for pi in range(2):
            nc.sync.dma_start(s64pp[pi][0:SBC, :], S64c_d[:])
        bmsgrow = const.tile([1, D], bft)
        nc.sync.dma_start(bmsgrow[:], bmsgrow_d[:])
        ones164 = const.tile([1, SBC], bft)
        nc.sync.dma_start(ones164[:], ones164_d[:])
        # cpm ping-pong tiles: rows 0..63 = cur@Wm1 + b_msg per cell
        # (rewritten per superblock), row 64 = -64*ones (loaded once).
        # Paired with the staircase tile (rows 0..63 = cell indicator,
        # row 64 = antimask in {0,1}) the stair matmul adds the per-cell
        # message term AND a -64 penalty on non-functional edges, which
        # the relu turns into exact zeros - no per-edge masking needed.
        cpm_pp = []
        for pi in range(2):
            t_ = const.tile([SBC + 1, D], bft, tag=f"cpm{pi}")
            nc.sync.dma_start(t_[SBC:SBC + 1, :], bigrow_d[:])
            cpm_pp.append(t_)

        aggldT = big.tile([128, NSB * 128], bft)   # col t*128 + 2c + m
        aggfT = big.tile([128, NSB * SBC], bft)    # col t*64 + c
        localT = big.tile([128, NS], bft)
        funcT = big.tile([128, NS], bft)

        def make_cpm(t):
            dst = cpm_pp[t % 2]
            pc = ps.tile([SBC, D], f32, tag="p")
            nc.tensor.matmul(pc[:], curTb[:, t * SBC:(t + 1) * SBC],
                             wt["Wm1"][:], start=True, stop=False)
            nc.tensor.matmul(pc[:], ones164[:], bmsgrow[:],
                             start=False, stop=True)
            nc.scalar.copy(dst[0:SBC, :], pc[:])
            return dst

        cpm_next = make_cpm(0)

        # remaining constants (needed mid-superblock-0 or later)
        blds = const.tile([128, NSUBT * 12], bft)
        nc.sync.dma_start(blds[:], Blds_d[:])
        invf = const.tile([128, NS], bft)
        nc.sync.dma_start(invf[:], invf_d[:])
        for k in ("Wl1", "Wl2", "Wu1", "Wu2", "Wc1", "Wc2"):
            t = const.tile([D, D], bft, tag=k)
            nc.sync.dma_start(t[:], W[k][:])
            wt[k] = t
        wg1 = const.tile([D, HG], bft)
        nc.sync.dma_start(wg1[:], W["Wg1"][:])
        wg2 = const.tile([HG, 3], bft)
        nc.sync.dma_start(wg2[:], W["Wg2"][:])
        bias = {}
        for k, ap in bias_in.items():
            t = const.tile(list(ap.shape), f32, tag=k)
            nc.sync.dma_start(t[:], ap[:])
            bias[k] = t
        oh3 = const.tile([3, 384], bft)
        nc.sync.dma_start(oh3[:], OH3_d[:])
        ones3 = const.tile([3, 1], bft)
        nc.sync.dma_start(ones3[:], ONES3_d[:])

        for t in range(NSB):
            cpm_t = cpm_next
            cb_t = combo0 if t == 0 else stream_in(t)
            natT_t = cb_t[:, 0:EPBF]
            nat_t = cb_t[:, EPBF:CPSB]
            s64_t = s64pp[t % 2]

            # local/distant aggregation matmuls are SPLIT around the
            # proj/stair epochs: the LDW-heavy (array-idle) agg stretch
            # would otherwise keep every 3.4us HAM window under the
            # clock-gate busy threshold; two short sparse stretches
            # between dense 512-col epochs raise the window-busy floor.
            def agg_half(rng, pagg_):
                for sl_ in rng:
                    s = t * NSUB + sl_
                    cb = CB_LOC[sl_]
                    w2 = 2 * min(6, SBC - cb)
                    nc.tensor.matmul(pagg_[:, 2 * cb:2 * cb + w2],
                                     nat_t[:, sl_ * 128:(sl_ + 1) * 128],
                                     blds[:, s * 12:s * 12 + w2],
                                     start=False, stop=(sl_ == NSUB - 1))

            # messages (transposed, sorted f-slots + penalty):
            # msgsT = relu(Wm2.T @ natTs + cpm @ stair - 64*antimask)
            msgsT = work.tile([128, EPBF], bft, tag="msgs")
            pagg = psagg.tile([128, 128], f32, tag="pagg")
            nc.vector.memset(pagg[:], 0.0)
            pqs = []
            for q, (q0, qn) in enumerate(FQ):
                pq = ps.tile([128, 512], f32, tag="p")
                pqs.append(pq)
                mm = nc.tensor.matmul(pq[:, 0:qn], wt["Wm2"][:],
                                      natT_t[:, q0:q0 + qn],
                                      start=True, stop=False)
                if q > 0:
                    mm.ins.ldweights = False
            agg_half(range(0, 6), pagg)
            for q, (q0, qn) in enumerate(FQ):
                mm = nc.tensor.matmul(pqs[q][:, 0:qn], cpm_t[:],
                                      s64_t[:, q0:q0 + qn],
                                      start=False, stop=True)
                if q > 0:
                    mm.ins.ldweights = False
                if q < 2:
                    nc.scalar.activation(msgsT[:, q0:q0 + qn],
                                         pqs[q][:, 0:qn], AF.Relu)
                else:
                    nc.vector.tensor_scalar(msgsT[:, q0:q0 + qn],
                                            pqs[q][:, 0:qn], 0.0, None,
                                            OP.max)
            agg_half(range(6, NSUB), pagg)
            nc.scalar.copy(aggldT[:, t * 128:(t + 1) * 128], pagg[:])

            # functional aggregation: segmented sum over the SLOT axis,
            # split as a gpsimd pairwise add (20 -> 10) + a DVE reduce
            # (10 -> 1), then per-cell 1/cnt_f scaling
            mv = msgsT[:].rearrange("p (c k) -> p c k", k=SLOT)
            r1 = work.tile([128, SBC, SLOT // 2], bft, tag="r1")
            nc.gpsimd.tensor_tensor(r1[:], mv[:, :, 0:SLOT // 2],
                                    mv[:, :, SLOT // 2:SLOT], OP.add)
            af = work.tile([128, SBC], f32, tag="af")
            nc.vector.tensor_reduce(af[:], r1[:], AX.X, OP.add)
            nc.vector.tensor_tensor(aggfT[:, t * SBC:(t + 1) * SBC], af[:],
                                    invf[:, t * SBC:(t + 1) * SBC], OP.mult)

            if t + 1 < NSB:
                cpm_next = make_cpm(t + 1)

        # ---------- second stage (transposed, chunked) ----------
        # order: gating first (Relu table is already loaded from the main
        # loop), then local/func (Tanh), then CNF with the final mix fused
        # into the last Euler step.
        curTf = const.tile([D, NS], f32)
        nc.sync.dma_start(curTf[:], curTf_d[:])

        def agg_view(off, ch):
            v = aggldT[:, ch * SB_PER_CHUNK * 128 + off:
                       (ch + 1) * SB_PER_CHUNK * 128:2]
            return v.rearrange("p (t c) -> p t c", c=SBC)

        hTg = big.tile([HG, NS], bft)
        for ch in range(NCHUNK):
            sl = slice(ch * CHUNK, (ch + 1) * CHUNK)
            ph = ps.tile([HG, CHUNK], f32, tag="p")
            mm = nc.tensor.matmul(ph[:], wg1[:], curTb[:, sl], start=True,
                                  stop=True)
            if ch > 0:
                mm.ins.ldweights = False
            nc.scalar.activation(hTg[:, sl], ph[:], AF.Relu,
                                 bias=bias["b_g1"][:])
        e3b = big.tile([3, NS], bft)
        for ch in range(NCHUNK):
            sl = slice(ch * CHUNK, (ch + 1) * CHUNK)
            pz = ps.tile([3, CHUNK], f32, tag="p")
            mm = nc.tensor.matmul(pz[:], wg2[:], hTg[:, sl], start=True,
                                  stop=True)
            if ch > 0:
                mm.ins.ldweights = False
            nc.scalar.activation(e3b[:, sl], pz[:], AF.Exp,
                                 bias=bias["b_g2"][:])
        lnf = big.tile([1, NS], f32)
        for ch in range(NCHUNK):
            sl = slice(ch * CHUNK, (ch + 1) * CHUNK)
            psum1 = ps.tile([1, CHUNK], f32, tag="p")
            mm = nc.tensor.matmul(psum1[:], ones3[:], e3b[:, sl], start=True,
                                  stop=True)
            if ch > 0:
                mm.ins.ldweights = False
            nc.scalar.activation(lnf[:, sl], psum1[:], AF.Ln)
        recf = big.tile([1, NS], f32)
        nc.scalar.activation(recf[:], lnf[:], AF.Exp, scale=-1.0)
        # normalized gates: g_m = e_m / den, in bf16, broadcast via PE
        rec3 = big.tile([3, NS], f32)
        nc.gpsimd.partition_broadcast(rec3[:], recf[:])
        g3b = big.tile([3, NS], bft)
        nc.vector.tensor_tensor(g3b[:], e3b[:], rec3[:], OP.mult)

        for ch in range(NCHUNK):
            sl = slice(ch * CHUNK, (ch + 1) * CHUNK)
            pl = ps.tile([128, CHUNK], f32, tag="p")
            nc.tensor.matmul(pl[:], wt["Wl1"][:], curTb[:, sl], start=True,
                             stop=False)
            nc.tensor.matmul(
                pl[:].rearrange("p (t c) -> p t c", c=SBC),
                wt["Wl2"][:], agg_view(0, ch), start=False, stop=True)
            nc.scalar.activation(localT[:, sl], pl[:], AF.Tanh,
                                 bias=bias["b_local"][:])
            pf = ps.tile([128, CHUNK], f32, tag="p")
            nc.tensor.matmul(pf[:], wt["Wu1"][:], curTb[:, sl], start=True,
                             stop=False)
            nc.tensor.matmul(pf[:], wt["Wu2"][:], aggfT[:, sl],
                             start=False, stop=True)
            nc.scalar.activation(funcT[:, sl], pf[:], AF.Tanh,
                                 bias=bias["b_upd"][:])

        # CNF: 3 Euler steps; final mix fused into the last step
        s_prev = curTf
        s_prev_bf = curTb
        for step in range(CNF_STEPS):
            last = step == CNF_STEPS - 1
            s_next = big.tile([128, NS], f32, tag=f"s{step % 2}")
            nb_next = None if last else big.tile([128, NS], bft,
                                                 tag=f"sbf{step % 2}")
            for ch in range(NCHUNK):
                sl = slice(ch * CHUNK, (ch + 1) * CHUNK)
                pp = ps.tile([128, CHUNK], f32, tag="p")
                nc.tensor.matmul(pp[:], wt["Wc1"][:], s_prev_bf[:, sl],
                                 start=True, stop=False)
                nc.tensor.matmul(
                    pp[:].rearrange("p (t c) -> p t c", c=SBC),
                    wt["Wc2"][:], agg_view(1, ch), start=False, stop=True)
                # state kept as v = 10*s: the Euler dt folds into the
                # bf16 re-scale copy (scale=0.1) and the gate constant
                th = temp1.tile([128, CHUNK], f32, tag="th")
                nc.scalar.activation(th[:], pp[:], AF.Tanh,
                                     bias=bias["b_cnf"][:])
                nc.vector.tensor_tensor(s_next[:, sl], s_prev[:, sl],
                                        th[:], OP.add)
                if not last:
                    nc.scalar.activation(nb_next[:, sl], s_next[:, sl],
                                         AF.Identity, scale=DTC)
                if last:
                    # final mix for this chunk
                    pe = []
                    for m in range(3):
                        p = ps.tile([128, CHUNK], f32, tag="p")
                        nc.tensor.matmul(p[:],
                                         oh3[:, m * 128:(m + 1) * 128],
                                         g3b[:, sl], start=True, stop=True)
                        pe.append(p)
                    acc = temp1.tile([128, CHUNK], f32, tag="acc")
                    tmp = temp1.tile([128, CHUNK], f32, tag="tmp")
                    nc.vector.tensor_tensor(acc[:], localT[:, sl],
                                            pe[0][:], OP.mult)
                    nc.vector.tensor_tensor(tmp[:], funcT[:, sl],
                                            pe[1][:], OP.mult)
                    nc.vector.tensor_tensor(acc[:], acc[:], tmp[:], OP.add)
                    nc.vector.tensor_tensor(tmp[:], s_next[:, sl],
                                            pe[2][:], OP.mult)
                    nc.vector.tensor_tensor(acc[:], acc[:], tmp[:], OP.add)
                    nc.sync.dma_start(outT[:, sl], acc[:])
            s_prev = s_next
            s_prev_bf = nb_next if step < CNF_STEPS - 1 else None

    nc.compile()
    return nc


_NC_CACHE = None


def _get_nc():
    global _NC_CACHE
    if _NC_CACHE is None:
        _NC_CACHE = _build_bass()
    return _NC_CACHE


def _prep_core_inputs(cur, nbr, conn, weights):
    """cur [NS, D] f32, nbr [NS, K, D] f32, conn [NS, K] i32 -> input map."""
    m = {}
    nf = nbr.reshape(E, D).astype(np.float32)
    connf = conn.reshape(E)
    cellof = np.arange(E) // K
    masks = [(connf == 0), (connf == 2), (connf == 1)]   # l, d, f
    cnts = [np.maximum(mk.reshape(NS, K).sum(1), 1).astype(np.float32)
            for mk in masks]
    # per-edge weights mask/cnt for local/distant
    wl_e = masks[0] / cnts[0][cellof]
    wd_e = masks[1] / cnts[1][cellof]

    # sorted f-path: f-edges first within each cell, truncated to SLOT
    # (exact whenever no cell has more than SLOT functional edges)
    nf_raw = masks[2].reshape(NS, K).sum(1)
    assert nf_raw.max() <= SLOT, f"nf overflow: {nf_raw.max()} > {SLOT}"
    order = np.argsort(~masks[2].reshape(NS, K), axis=1,
                       kind="stable")[:, :SLOT]           # [NS, SLOT]
    natTs = nbr.reshape(NS, K, D)[np.arange(NS)[:, None], order]
    natTs_b = natTs.reshape(NS * SLOT, D).T.reshape(D, NSB, EPBF)
    # combo stream: per superblock [sorted natT block | natural block]
    nat_b = (nf.reshape(NSUBT, 128, D).transpose(1, 0, 2)
             .reshape(128, NSB, EPB))                     # [p, t, (s,d)]
    m["combo"] = np.ascontiguousarray(
        np.concatenate([natTs_b, nat_b], axis=2).reshape(128, NSB * CPSB)
    ).astype(bf16)
    # antimask row: 1.0 on padding f-slots (pairs with the -64 row)
    m["antimask"] = (np.arange(SLOT)[None, :] >= nf_raw[:, None]) \
        .reshape(1, EF).astype(bf16)
    # 1/cnt_f per cell, replicated to 128 partitions
    m["invf_bc"] = np.broadcast_to(
        (1.0 / cnts[2])[None, :], (128, NS)).astype(bf16)
    m["bigrow"] = np.full((1, D), -64.0, np.float32).astype(bf16)

    # B_lds [128, NSUBT*12]: col s*12 + 2*(c_local-cb) + m, pre-scaled
    blds = np.zeros((128, NSUBT * 12), np.float32)
    e_idx = np.arange(E)
    s_idx = e_idx // 128
    p_idx = e_idx % 128
    cb_s = np.array([CB_LOC[si % NSUB] for si in range(NSUBT)])[s_idx]
    j2 = (cellof % SBC) - cb_s
    for mi, we in ((0, wl_e), (1, wd_e)):
        blds[p_idx, s_idx * 12 + 2 * j2 + mi] = we
    m["B_lds"] = blds.astype(bf16)

    ct = np.ascontiguousarray(cur.T)
    m["curT_f"] = (ct * 10.0).astype(np.float32)   # CNF state as 10*s
    m["curT_b"] = ct.astype(bf16)

    Wl, Wm, Wu, Wc = (weights["W_local"], weights["W_msg"],
                      weights["W_upd"], weights["W_cnf"])
    m["Wl1"], m["Wl2"] = Wl[:D].astype(bf16), Wl[D:].astype(bf16)
    m["Wm1"], m["Wm2"] = Wm[:D].astype(bf16), Wm[D:].astype(bf16)
    m["Wu1"], m["Wu2"] = Wu[:D].astype(bf16), Wu[D:].astype(bf16)
    m["Wc1"], m["Wc2"] = Wc[:D].astype(bf16), Wc[D:].astype(bf16)
    m["Wg1"] = weights["W_g1"].astype(bf16)
    m["Wg2"] = weights["W_g2"].astype(bf16)
    m["b_msg_row"] = weights["b_msg"].reshape(1, D).astype(bf16)
    m["b_local"] = weights["b_local"].reshape(D, 1).astype(np.float32)
    m["b_upd"] = weights["b_upd"].reshape(D, 1).astype(np.float32)
    m["b_cnf"] = weights["b_cnf"].reshape(D, 1).astype(np.float32)
    m["b_g1"] = weights["b_g1"].reshape(HG, 1).astype(np.float32)
    m["b_g2"] = weights["b_g2"].reshape(3, 1).astype(np.float32)
    for k, v in CONSTS.items():
        m[k] = v
    return m


def kernel(**inputs):
    from concourse.bass_utils import run_bass_kernel_spmd

    cur = np.asarray(inputs["current_state"], np.float32)
    nbr = np.asarray(inputs["neighbor_states"], np.float32)
    conn = np.asarray(inputs["conn_type"], np.int32)
    weights = {k: np.asarray(v, np.float32) for k, v in inputs.items()
               if k not in ("current_state", "neighbor_states", "conn_type")}

    npad = NCORES * NS
    cur_p = np.zeros((npad, D), np.float32)
    cur_p[:N_CELLS] = cur
    nbr_p = np.zeros((npad, K, D), np.float32)
    nbr_p[:N_CELLS] = nbr
    conn_p = np.full((npad, K), 3, np.int32)
    conn_p[:N_CELLS] = conn

    in_maps = []
    for c in range(NCORES):
        sl = slice(c * NS, (c + 1) * NS)
        in_maps.append(_prep_core_inputs(cur_p[sl], nbr_p[sl], conn_p[sl],
                                         weights))
    nc = _get_nc()
    res = run_bass_kernel_spmd(nc, in_maps, list(range(NCORES)))
    out = np.concatenate([res.results[c]["outT"].T for c in range(NCORES)],
                         axis=0)
    return np.ascontiguousarray(out[:N_CELLS]).astype(np.float32)


if __name__ == "__main__":
    pass


# revision 71
# speedup vs baseline: 1.1952x; 1.0176x over previous
"""Trainium2 Bass kernel for nn_MoEConnectionProcessor (v2).

Data-parallel over cells: 8 cores x 2560 padded cells (19683 real).
Per core: 40 superblocks of 64 cells (1664 edges each).

v2 design (vs v1): minimize PE instruction count / stationary swaps.
  - message projection runs TRANSPOSED: stationary Wm2 (one LDW per
    superblock), moving operand = host-pretransposed nbr^T, pre-masked
    by the functional mask and pre-scaled by 1/cnt_f (relu is positive
    homogeneous, so the scaling commutes through relu).
  - the per-cell term (cur @ Wm1 + b_msg) is added into the same PSUM
    via a 65-row matmul: rows 0..63 = cpm per cell, row 64 = b_msg;
    moving operand = masked staircase built on-device from a host
    weight row (gpsimd partition-broadcast + DVE multiply).
  - functional aggregation = DVE segmented reduce over the 26-edge
    axis of the relu'd transposed messages (no matmul, no masks).
  - local/distant aggregation stays on PE (per-subtile stationary) but
    with host-prebuilt mask*staircase*(1/cnt) moving columns, so
    counts, reciprocals, and mask building all disappear from device.
  - all DMA is contiguous (no DMA-transpose): host prepares both
    layouts of neighbor data.
"""

import numpy as np
import ml_dtypes
from contextlib import ExitStack

N_CELLS, K, D, HG = 19683, 26, 128, 64
NCORES = 8
NS = 2560                 # padded cells per core
SBC = 64                  # cells per superblock
NSB = NS // SBC           # 40 superblocks
NSUB = 13                 # subtiles (128 edges) per superblock
EPB = NSUB * 128          # 1664 edges per superblock
E = NS * K                # 66560 edges per core
NSUBT = NS * K // 128     # 520 subtiles per core
SLOT = 20                 # f-edge slots per cell (max nf in the input)
EPBF = SBC * SLOT         # 1280 f-path edges per superblock
EF = NS * SLOT            # f-path edges per core
FQ = [(0, 512), (512, 512), (1024, 256)]   # f-psum chunks
CPSB = EPBF + EPB         # combo cols per superblock
CHUNK = 512
NCHUNK = NS // CHUNK      # 5
SB_PER_CHUNK = CHUNK // SBC  # 8
CNF_STEPS, DTC = 3, 0.1

bf16 = ml_dtypes.bfloat16

# first local cell of each subtile class (within a 64-cell superblock)
CB_LOC = [(chi * 128) // K for chi in range(NSUB)]


def _consts():
    c = {}
    # S64c [64, EPBF]: staircase indicator for the sorted f-path,
    # cell = e // SLOT (same for all superblocks)
    s64 = np.zeros((SBC, EPBF), np.float32)
    s64[np.arange(EPBF) // SLOT, np.arange(EPBF)] = 1.0
    c["S64c"] = s64.astype(bf16)
    oh = np.zeros((3, 3 * 128), np.float32)
    for m in range(3):
        # distant expert's state is carried as v = 10*s; its gate block
        # absorbs the 0.1
        oh[m, m * 128:(m + 1) * 128] = 0.1 if m == 2 else 1.0
    c["OH3"] = oh.astype(bf16)
    c["ONES3"] = np.ones((3, 1), np.float32).astype(bf16)
    c["ONES164"] = np.ones((1, SBC), np.float32).astype(bf16)
    return c


CONSTS = _consts()


def _build_bass():
    import concourse.bass as bass
    import concourse.tile as tile
    from concourse import bacc, mybir

    f32, bft, i32 = mybir.dt.float32, mybir.dt.bfloat16, mybir.dt.int32
    AF = mybir.ActivationFunctionType
    OP = mybir.AluOpType
    AX = mybir.AxisListType

    nc = bacc.Bacc("TRN2", target_bir_lowering=False, debug=False,
                   num_devices=NCORES)

    def din(name, shape, dt):
        return nc.dram_tensor(name, shape, dt, kind="ExternalInput").ap()

    combo_d = din("combo", [128, NSB * CPSB], bft)  # [natTs | nat] per sb
    anti_d = din("antimask", [1, EF], bft)       # 1.0 on padding f-slots
    invf_d = din("invf_bc", [128, NS], bft)      # 1/cnt_f row-replicated
    Blds_d = din("B_lds", [128, NSUBT * 12], bft)
    S64c_d = din("S64c", [SBC, EPBF], bft)
    bmsgrow_d = din("b_msg_row", [1, D], bft)
    ones164_d = din("ONES164", [1, SBC], bft)
    bigrow_d = din("bigrow", [1, D], bft)        # all-ones row
    curTb_d = din("curT_b", [D, NS], bft)
    curTf_d = din("curT_f", [D, NS], f32)
    wnames = ["Wl1", "Wl2", "Wm1", "Wm2", "Wu1", "Wu2", "Wc1", "Wc2"]
    W = {k: din(k, [D, D], bft) for k in wnames}
    W["Wg1"] = din("Wg1", [D, HG], bft)
    W["Wg2"] = din("Wg2", [HG, 3], bft)
    bias_in = {
        "b_local": din("b_local", [D, 1], f32),
        "b_upd": din("b_upd", [D, 1], f32),
        "b_cnf": din("b_cnf", [D, 1], f32),
        "b_g1": din("b_g1", [HG, 1], f32),
        "b_g2": din("b_g2", [3, 1], f32),
    }
    OH3_d = din("OH3", [3, 384], bft)
    ONES3_d = din("ONES3", [3, 1], bft)
    outT = nc.dram_tensor("outT", [D, NS], f32, kind="ExternalOutput").ap()

    with tile.TileContext(nc) as tc, ExitStack() as ctx:
        const = ctx.enter_context(tc.tile_pool(name="const", bufs=1))
        big = ctx.enter_context(tc.tile_pool(name="big", bufs=1))
        stream = ctx.enter_context(tc.tile_pool(name="stream", bufs=4))
        work = ctx.enter_context(tc.tile_pool(name="work", bufs=2))
        temp1 = ctx.enter_context(tc.tile_pool(name="temp1", bufs=3))
        ps = ctx.enter_context(tc.tile_pool(name="ps", bufs=6, space="PSUM"))
        psagg = ctx.enter_context(tc.tile_pool(name="psagg", bufs=2,
                                               space="PSUM"))

        # ---------- load constants / weights ----------
        # DMA order matters: combo0 (warm-up gate + superblock 0's data)
        # is the very first trigger; everything else loads behind it.
        s64pp = []
        for pi in range(2):
            t_ = const.tile([SBC + 1, EPBF], bft, tag=f"s64pp{pi}")
            s64pp.append(t_)

        def stream_in(t):
            cb_ = stream.tile([128, CPSB], bft, tag="combo")
            nc.sync.dma_start(cb_[:],
                              combo_d[:, t * CPSB:(t + 1) * CPSB])
            nc.sync.dma_start(s64pp[t % 2][SBC:SBC + 1, :],
                              anti_d[:, t * EPBF:(t + 1) * EPBF])
            return cb_

        combo0 = stream_in(0)
        # PE warm-up, gated on the first stream tile so it runs right
        # before superblock 0: ~14 back-to-back 512-col matmuls keep the
        # PE busy >3.4us continuously, opening the HAM clock gate
        # (1.2 -> 2.4 GHz).
        for i in range(14):
            pwu = ps.tile([128, CHUNK], f32, tag="p")
            mm = nc.tensor.matmul(pwu[:], combo0[:, 0:128],
                                  combo0[:, 0:CHUNK], start=True, stop=True)
            if i > 0:
                mm.ins.ldweights = False

        wt = {}
        for k in ("Wm1", "Wm2"):
            t = const.tile([D, D], bft, tag=k)
            nc.sync.dma_start(t[:], W[k][:])
            wt[k] = t
        curTb = const.tile([D, NS], bft)
        nc.sync.dma_start(curTb[:], curTb_d[:])
        for pi in range(2):
            nc.sync.dma_start(s64pp[pi][0:SBC, :], S64c_d[:])
        bmsgrow = const.tile([1, D], bft)
        nc.sync.dma_start(bmsgrow[:], bmsgrow_d[:])
        ones164 = const.tile([1, SBC], bft)
        nc.sync.dma_start(ones164[:], ones164_d[:])
        # cpm ping-pong tiles: rows 0..63 = cur@Wm1 + b_msg per cell
        # (rewritten per superblock), row 64 = -64*ones (loaded once).
        # Paired with the staircase tile (rows 0..63 = cell indicator,
        # row 64 = antimask in {0,1}) the stair matmul adds the per-cell
        # message term AND a -64 penalty on non-functional edges, which
        # the relu turns into exact zeros - no per-edge masking needed.
        cpm_pp = []
        for pi in range(2):
            t_ = const.tile([SBC + 1, D], bft, tag=f"cpm{pi}")
            nc.sync.dma_start(t_[SBC:SBC + 1, :], bigrow_d[:])
            cpm_pp.append(t_)

        aggldT = big.tile([128, NSB * 128], bft)   # col t*128 + 2c + m
        aggfT = big.tile([128, NSB * SBC], bft)    # col t*64 + c
        localT = big.tile([128, NS], bft)
        funcT = big.tile([128, NS], bft)

        def make_cpm(t):
            dst = cpm_pp[t % 2]
            pc = ps.tile([SBC, D], f32, tag="p")
            nc.tensor.matmul(pc[:], curTb[:, t * SBC:(t + 1) * SBC],
                             wt["Wm1"][:], start=True, stop=False)
            nc.tensor.matmul(pc[:], ones164[:], bmsgrow[:],
                             start=False, stop=True)
            nc.scalar.copy(dst[0:SBC, :], pc[:])
            return dst

        cpm_next = make_cpm(0)

        # remaining constants (needed mid-superblock-0 or later)
        blds = const.tile([128, NSUBT * 12], bft)
        nc.sync.dma_start(blds[:], Blds_d[:])
        invf = const.tile([128, NS], bft)
        nc.sync.dma_start(invf[:], invf_d[:])
        for k in ("Wl1", "Wl2", "Wu1", "Wu2", "Wc1", "Wc2"):
            t = const.tile([D, D], bft, tag=k)
            nc.sync.dma_start(t[:], W[k][:])
            wt[k] = t
        wg1 = const.tile([D, HG], bft)
        nc.sync.dma_start(wg1[:], W["Wg1"][:])
        wg2 = const.tile([HG, 3], bft)
        nc.sync.dma_start(wg2[:], W["Wg2"][:])
        bias = {}
        for k, ap in bias_in.items():
            t = const.tile(list(ap.shape), f32, tag=k)
            nc.sync.dma_start(t[:], ap[:])
            bias[k] = t
        oh3 = const.tile([3, 384], bft)
        nc.sync.dma_start(oh3[:], OH3_d[:])
        ones3 = const.tile([3, 1], bft)
        nc.sync.dma_start(ones3[:], ONES3_d[:])

        for t in range(NSB):
            cpm_t = cpm_next
            cb_t = combo0 if t == 0 else stream_in(t)
            natT_t = cb_t[:, 0:EPBF]
            nat_t = cb_t[:, EPBF:CPSB]
            s64_t = s64pp[t % 2]

            # local/distant aggregation matmuls are SPLIT around the
            # proj/stair epochs: the LDW-heavy (array-idle) agg stretch
            # would otherwise keep every 3.4us HAM window under the
            # clock-gate busy threshold; two short sparse stretches
            # between dense 512-col epochs raise the window-busy floor.
            def agg_half(rng, pagg_):
                for sl_ in rng:
                    s = t * NSUB + sl_
                    cb = CB_LOC[sl_]
                    w2 = 2 * min(6, SBC - cb)
                    nc.tensor.matmul(pagg_[:, 2 * cb:2 * cb + w2],
                                     nat_t[:, sl_ * 128:(sl_ + 1) * 128],
                                     blds[:, s * 12:s * 12 + w2],
                                     start=False, stop=(sl_ == NSUB - 1))

            # messages (transposed, sorted f-slots + penalty):
            # msgsT = relu(Wm2.T @ natTs + cpm @ stair - 64*antimask)
            msgsT = work.tile([128, EPBF], bft, tag="msgs")
            pagg = psagg.tile([128, 128], f32, tag="pagg")
            nc.vector.memset(pagg[:], 0.0)
            pqs = []
            for q, (q0, qn) in enumerate(FQ):
                pq = ps.tile([128, 512], f32, tag="p")
                pqs.append(pq)
                mm = nc.tensor.matmul(pq[:, 0:qn], wt["Wm2"][:],
                                      natT_t[:, q0:q0 + qn],
                                      start=True, stop=False)
                if q > 0:
                    mm.ins.ldweights = False
            agg_half(range(0, 8), pagg)
            for q, (q0, qn) in enumerate(FQ):
                mm = nc.tensor.matmul(pqs[q][:, 0:qn], cpm_t[:],
                                      s64_t[:, q0:q0 + qn],
                                      start=False, stop=True)
                if q > 0:
                    mm.ins.ldweights = False
                if q < 2:
                    nc.scalar.activation(msgsT[:, q0:q0 + qn],
                                         pqs[q][:, 0:qn], AF.Relu)
                else:
                    nc.vector.tensor_scalar(msgsT[:, q0:q0 + qn],
                                            pqs[q][:, 0:qn], 0.0, None,
                                            OP.max)
            agg_half(range(8, NSUB), pagg)
            nc.scalar.copy(aggldT[:, t * 128:(t + 1) * 128], pagg[:])

            # functional aggregation: segmented sum over the SLOT axis,
            # split as a gpsimd pairwise add (20 -> 10) + a DVE reduce
            # (10 -> 1), then per-cell 1/cnt_f scaling
            mv = msgsT[:].rearrange("p (c k) -> p c k", k=SLOT)
            r1 = work.tile([128, SBC, SLOT // 2], bft, tag="r1")
            nc.gpsimd.tensor_tensor(r1[:], mv[:, :, 0:SLOT // 2],
                                    mv[:, :, SLOT // 2:SLOT], OP.add)
            af = work.tile([128, SBC], f32, tag="af")
            nc.vector.tensor_reduce(af[:], r1[:], AX.X, OP.add)
            nc.vector.tensor_tensor(aggfT[:, t * SBC:(t + 1) * SBC], af[:],
                                    invf[:, t * SBC:(t + 1) * SBC], OP.mult)

            if t + 1 < NSB:
                cpm_next = make_cpm(t + 1)

        # ---------- second stage (transposed, chunked) ----------
        # order: gating first (Relu table is already loaded from the main
        # loop), then local/func (Tanh), then CNF with the final mix fused
        # into the last Euler step.
        curTf = const.tile([D, NS], f32)
        nc.sync.dma_start(curTf[:], curTf_d[:])

        def agg_view(off, ch):
            v = aggldT[:, ch * SB_PER_CHUNK * 128 + off:
                       (ch + 1) * SB_PER_CHUNK * 128:2]
            return v.rearrange("p (t c) -> p t c", c=SBC)

        hTg = big.tile([HG, NS], bft)
        for ch in range(NCHUNK):
            sl = slice(ch * CHUNK, (ch + 1) * CHUNK)
            ph = ps.tile([HG, CHUNK], f32, tag="p")
            mm = nc.tensor.matmul(ph[:], wg1[:], curTb[:, sl], start=True,
                                  stop=True)
            if ch > 0:
                mm.ins.ldweights = False
            nc.scalar.activation(hTg[:, sl], ph[:], AF.Relu,
                                 bias=bias["b_g1"][:])
        e3b = big.tile([3, NS], bft)
        for ch in range(NCHUNK):
            sl = slice(ch * CHUNK, (ch + 1) * CHUNK)
            pz = ps.tile([3, CHUNK], f32, tag="p")
            mm = nc.tensor.matmul(pz[:], wg2[:], hTg[:, sl], start=True,
                                  stop=True)
            if ch > 0:
                mm.ins.ldweights = False
            nc.scalar.activation(e3b[:, sl], pz[:], AF.Exp,
                                 bias=bias["b_g2"][:])
        lnf = big.tile([1, NS], f32)
        for ch in range(NCHUNK):
            sl = slice(ch * CHUNK, (ch + 1) * CHUNK)
            psum1 = ps.tile([1, CHUNK], f32, tag="p")
            mm = nc.tensor.matmul(psum1[:], ones3[:], e3b[:, sl], start=True,
                                  stop=True)
            if ch > 0:
                mm.ins.ldweights = False
            nc.scalar.activation(lnf[:, sl], psum1[:], AF.Ln)
        recf = big.tile([1, NS], f32)
        nc.scalar.activation(recf[:], lnf[:], AF.Exp, scale=-1.0)
        # normalized gates: g_m = e_m / den, in bf16, broadcast via PE
        rec3 = big.tile([3, NS], f32)
        nc.gpsimd.partition_broadcast(rec3[:], recf[:])
        g3b = big.tile([3, NS], bft)
        nc.vector.tensor_tensor(g3b[:], e3b[:], rec3[:], OP.mult)

        for ch in range(NCHUNK):
            sl = slice(ch * CHUNK, (ch + 1) * CHUNK)
            pl = ps.tile([128, CHUNK], f32, tag="p")
            nc.tensor.matmul(pl[:], wt["Wl1"][:], curTb[:, sl], start=True,
                             stop=False)
            nc.tensor.matmul(
                pl[:].rearrange("p (t c) -> p t c", c=SBC),
                wt["Wl2"][:], agg_view(0, ch), start=False, stop=True)
            nc.scalar.activation(localT[:, sl], pl[:], AF.Tanh,
                                 bias=bias["b_local"][:])
            pf = ps.tile([128, CHUNK], f32, tag="p")
            nc.tensor.matmul(pf[:], wt["Wu1"][:], curTb[:, sl], start=True,
                             stop=False)
            nc.tensor.matmul(pf[:], wt["Wu2"][:], aggfT[:, sl],
                             start=False, stop=True)
            nc.scalar.activation(funcT[:, sl], pf[:], AF.Tanh,
                                 bias=bias["b_upd"][:])

        # CNF: 3 Euler steps; final mix fused into the last step
        s_prev = curTf
        s_prev_bf = curTb
        for step in range(CNF_STEPS):
            last = step == CNF_STEPS - 1
            s_next = big.tile([128, NS], f32, tag=f"s{step % 2}")
            nb_next = None if last else big.tile([128, NS], bft,
                                                 tag=f"sbf{step % 2}")
            for ch in range(NCHUNK):
                sl = slice(ch * CHUNK, (ch + 1) * CHUNK)
                pp = ps.tile([128, CHUNK], f32, tag="p")
                nc.tensor.matmul(pp[:], wt["Wc1"][:], s_prev_bf[:, sl],
                                 start=True, stop=False)
                nc.tensor.matmul(
                    pp[:].rearrange("p (t c) -> p t c", c=SBC),
                    wt["Wc2"][:], agg_view(1, ch), start=False, stop=True)
                # state kept as v = 10*s: the Euler dt folds into the
                # bf16 re-scale copy (scale=0.1) and the gate constant
                th = temp1.tile([128, CHUNK], f32, tag="th")
                nc.scalar.activation(th[:], pp[:], AF.Tanh,
                                     bias=bias["b_cnf"][:])
                nc.vector.tensor_tensor(s_next[:, sl], s_prev[:, sl],
                                        th[:], OP.add)
                if not last:
                    nc.scalar.activation(nb_next[:, sl], s_next[:, sl],
                                         AF.Identity, scale=DTC)
                if last:
                    # final mix for this chunk
                    pe = []
                    for m in range(3):
                        p = ps.tile([128, CHUNK], f32, tag="p")
                        nc.tensor.matmul(p[:],
                                         oh3[:, m * 128:(m + 1) * 128],
                                         g3b[:, sl], start=True, stop=True)
                        pe.append(p)
                    acc = temp1.tile([128, CHUNK], f32, tag="acc")
                    tmp = temp1.tile([128, CHUNK], f32, tag="tmp")
                    nc.vector.tensor_tensor(acc[:], localT[:, sl],
                                            pe[0][:], OP.mult)
                    nc.vector.tensor_tensor(tmp[:], funcT[:, sl],
                                            pe[1][:], OP.mult)
                    nc.vector.tensor_tensor(acc[:], acc[:], tmp[:], OP.add)
                    nc.vector.tensor_tensor(tmp[:], s_next[:, sl],
                                            pe[2][:], OP.mult)
                    nc.vector.tensor_tensor(acc[:], acc[:], tmp[:], OP.add)
                    nc.sync.dma_start(outT[:, sl], acc[:])
            s_prev = s_next
            s_prev_bf = nb_next if step < CNF_STEPS - 1 else None

    nc.compile()
    return nc


_NC_CACHE = None


def _get_nc():
    global _NC_CACHE
    if _NC_CACHE is None:
        _NC_CACHE = _build_bass()
    return _NC_CACHE


def _prep_core_inputs(cur, nbr, conn, weights):
    """cur [NS, D] f32, nbr [NS, K, D] f32, conn [NS, K] i32 -> input map."""
    m = {}
    nf = nbr.reshape(E, D).astype(np.float32)
    connf = conn.reshape(E)
    cellof = np.arange(E) // K
    masks = [(connf == 0), (connf == 2), (connf == 1)]   # l, d, f
    cnts = [np.maximum(mk.reshape(NS, K).sum(1), 1).astype(np.float32)
            for mk in masks]
    # per-edge weights mask/cnt for local/distant
    wl_e = masks[0] / cnts[0][cellof]
    wd_e = masks[1] / cnts[1][cellof]

    # sorted f-path: f-edges first within each cell, truncated to SLOT
    # (exact whenever no cell has more than SLOT functional edges)
    nf_raw = masks[2].reshape(NS, K).sum(1)
    assert nf_raw.max() <= SLOT, f"nf overflow: {nf_raw.max()} > {SLOT}"
    order = np.argsort(~masks[2].reshape(NS, K), axis=1,
                       kind="stable")[:, :SLOT]           # [NS, SLOT]
    natTs = nbr.reshape(NS, K, D)[np.arange(NS)[:, None], order]
    natTs_b = natTs.reshape(NS * SLOT, D).T.reshape(D, NSB, EPBF)
    # combo stream: per superblock [sorted natT block | natural block]
    nat_b = (nf.reshape(NSUBT, 128, D).transpose(1, 0, 2)
             .reshape(128, NSB, EPB))                     # [p, t, (s,d)]
    m["combo"] = np.ascontiguousarray(
        np.concatenate([natTs_b, nat_b], axis=2).reshape(128, NSB * CPSB)
    ).astype(bf16)
    # antimask row: 1.0 on padding f-slots (pairs with the -64 row)
    m["antimask"] = (np.arange(SLOT)[None, :] >= nf_raw[:, None]) \
        .reshape(1, EF).astype(bf16)
    # 1/cnt_f per cell, replicated to 128 partitions
    m["invf_bc"] = np.broadcast_to(
        (1.0 / cnts[2])[None, :], (128, NS)).astype(bf16)
    m["bigrow"] = np.full((1, D), -64.0, np.float32).astype(bf16)

    # B_lds [128, NSUBT*12]: col s*12 + 2*(c_local-cb) + m, pre-scaled
    blds = np.zeros((128, NSUBT * 12), np.float32)
    e_idx = np.arange(E)
    s_idx = e_idx // 128
    p_idx = e_idx % 128
    cb_s = np.array([CB_LOC[si % NSUB] for si in range(NSUBT)])[s_idx]
    j2 = (cellof % SBC) - cb_s
    for mi, we in ((0, wl_e), (1, wd_e)):
        blds[p_idx, s_idx * 12 + 2 * j2 + mi] = we
    m["B_lds"] = blds.astype(bf16)

    ct = np.ascontiguousarray(cur.T)
    m["curT_f"] = (ct * 10.0).astype(np.float32)   # CNF state as 10*s
    m["curT_b"] = ct.astype(bf16)

    Wl, Wm, Wu, Wc = (weights["W_local"], weights["W_msg"],
                      weights["W_upd"], weights["W_cnf"])
    m["Wl1"], m["Wl2"] = Wl[:D].astype(bf16), Wl[D:].astype(bf16)
    m["Wm1"], m["Wm2"] = Wm[:D].astype(bf16), Wm[D:].astype(bf16)
    m["Wu1"], m["Wu2"] = Wu[:D].astype(bf16), Wu[D:].astype(bf16)
    m["Wc1"], m["Wc2"] = Wc[:D].astype(bf16), Wc[D:].astype(bf16)
    m["Wg1"] = weights["W_g1"].astype(bf16)
    m["Wg2"] = weights["W_g2"].astype(bf16)
    m["b_msg_row"] = weights["b_msg"].reshape(1, D).astype(bf16)
    m["b_local"] = weights["b_local"].reshape(D, 1).astype(np.float32)
    m["b_upd"] = weights["b_upd"].reshape(D, 1).astype(np.float32)
    m["b_cnf"] = weights["b_cnf"].reshape(D, 1).astype(np.float32)
    m["b_g1"] = weights["b_g1"].reshape(HG, 1).astype(np.float32)
    m["b_g2"] = weights["b_g2"].reshape(3, 1).astype(np.float32)
    for k, v in CONSTS.items():
        m[k] = v
    return m


def kernel(**inputs):
    from concourse.bass_utils import run_bass_kernel_spmd

    cur = np.asarray(inputs["current_state"], np.float32)
    nbr = np.asarray(inputs["neighbor_states"], np.float32)
    conn = np.asarray(inputs["conn_type"], np.int32)
    weights = {k: np.asarray(v, np.float32) for k, v in inputs.items()
               if k not in ("current_state", "neighbor_states", "conn_type")}

    npad = NCORES * NS
    cur_p = np.zeros((npad, D), np.float32)
    cur_p[:N_CELLS] = cur
    nbr_p = np.zeros((npad, K, D), np.float32)
    nbr_p[:N_CELLS] = nbr
    conn_p = np.full((npad, K), 3, np.int32)
    conn_p[:N_CELLS] = conn

    in_maps = []
    for c in range(NCORES):
        sl = slice(c * NS, (c + 1) * NS)
        in_maps.append(_prep_core_inputs(cur_p[sl], nbr_p[sl], conn_p[sl],
                                         weights))
    nc = _get_nc()
    res = run_bass_kernel_spmd(nc, in_maps, list(range(NCORES)))
    out = np.concatenate([res.results[c]["outT"].T for c in range(NCORES)],
                         axis=0)
    return np.ascontiguousarray(out[:N_CELLS]).astype(np.float32)


if __name__ == "__main__":
    pass
